# revision 10
# baseline (speedup 1.0000x reference)
"""Trainium2 Bass kernel for a 2-layer GCN + link predictor (PrimeKG drug
repurposing GNN).

Strategy (8 NeuronCores, SPMD single program):
  - Nodes are permuted into 128-node "buckets" balanced by in-degree; each
    core owns NBLK/8 consecutive buckets (rows of the aggregation).
  - Edges are grouped by destination bucket and padded to chunks of 128.
    segment_sum is computed per bucket as a sequence of PE matmuls:
       aggT[f, n] += sum_e xgath[e, f] * onehot[e, n]
    where xgath is an indirect-DMA gather of source-node features and
    onehot[e, n] = vals[e] * (n == local_row[e]) built on the vector engine.
  - x = node_emb + type_onehot.T @ type_emb is built sharded (original node
    order, host-precomputed transposed one-hot) and AllGathered; h and z
    live in permuted order, sharded and AllGathered likewise.
  - Pairs are sharded by batch; the predictor gathers z rows, transposes on
    the PE, and runs the tiny MLP per 128-pair chunk.

Feature tables and matmuls are fp16 with f32 PSUM accumulation.
"""

import numpy as np

import concourse.bass as bass
import concourse.bacc as bacc
import concourse.tile as tile
import concourse.mybir as mybir
from concourse import bass_utils

PT = 128  # partitions
NCORES = 8
TE = 16   # padded type-embedding rows

F16 = np.float16

_prog_cache: dict = {}


def _preprocess(node_type_ids, adj_rows, adj_cols, adj_vals, pairs,
                node_emb, type_emb, W1, b1, W2, b2, Wp1, bp1, Wp2, bp2):
    N, H = node_emb.shape
    T = type_emb.shape[0]
    E = adj_rows.shape[0]
    P2 = pairs.shape[1]
    D = W2.shape[1]
    assert H == PT and T <= TE and Wp1.shape == (3 * D, D)

    NPB = PT * NCORES
    NPAD = -(-N // NPB) * NPB
    NBLK = NPAD // PT
    BPC = NBLK // NCORES

    rows = np.asarray(adj_rows).astype(np.int64)
    cols = np.asarray(adj_cols).astype(np.int64)
    vals = np.asarray(adj_vals).astype(np.float32)
    types = np.asarray(node_type_ids).astype(np.int64)

    # Degree-balanced bucket assignment: deal nodes (sorted by in-degree
    # desc) round-robin across the NBLK buckets, then repair-swap nodes
    # between heavy and light buckets to pull the max bucket load down to
    # the next multiple-of-128 boundary.
    deg = np.bincount(rows, minlength=N).astype(np.int64)
    deg_pad = np.zeros(NPAD, np.int64)
    deg_pad[:N] = deg
    order = np.argsort(-deg_pad, kind="stable")
    i = np.arange(NPAD)
    bucket_of_rank = i % NBLK
    slot_of_rank = i // NBLK
    bucket_of = np.empty(NPAD, np.int64)
    bucket_of[order] = bucket_of_rank
    loads = np.bincount(bucket_of, weights=deg_pad, minlength=NBLK).astype(
        np.int64)
    target = max(PT, int(-(-int(loads.max()) // PT) - 1) * PT)
    members = [list(order[b::NBLK][::-1]) for b in range(NBLK)]  # asc degree
    for _ in range(2000):
        hb = int(np.argmax(loads))
        if loads[hb] <= target:
            break
        lb = int(np.argmin(loads))
        done = False
        for mi in range(len(members[hb]) - 1, -1, -1):
            m = members[hb][mi]
            for li, l in enumerate(members[lb]):
                delta = deg_pad[m] - deg_pad[l]
                if delta <= 0:
                    break
                if loads[lb] + delta <= target:
                    members[hb][mi], members[lb][li] = l, m
                    loads[hb] -= delta
                    loads[lb] += delta
                    done = True
                    break
            if done:
                break
        if not done:
            break
    perm = np.empty(N, np.int64)
    for b in range(NBLK):
        for s, m in enumerate(members[b]):
            if m < N:
                perm[m] = b * PT + s

    prow = perm[rows]
    bkt = prow // PT
    rid = (prow % PT).astype(np.float32)
    cnt = np.bincount(bkt, minlength=NBLK)
    C = max(1, int(-(-int(cnt.max()) // PT)))
    CAP = C * PT

    eord = np.argsort(bkt, kind="stable")
    bs = bkt[eord]
    starts = np.concatenate([[0], np.cumsum(cnt)[:-1]])
    ps = np.arange(E) - starts[bs]

    ecol1 = np.zeros((NBLK, CAP), np.int32)
    ecol2 = np.zeros((NBLK, CAP), np.int32)
    erid = np.zeros((NBLK, CAP), np.float32)
    evalv = np.zeros((NBLK, CAP), np.float32)
    ce = cols[eord]
    ecol1[bs, ps] = ce
    ecol2[bs, ps] = perm[ce]
    erid[bs, ps] = rid[eord]
    evalv[bs, ps] = vals[eord]

    def per_core_T(a):
        # [NBLK, C*PT] -> per-core [PT, BPC*C]; column blk*C+c, partition p
        # holds bucket (core*BPC+blk) edge slot c*128+p.
        out = []
        for k in range(NCORES):
            sub = a[k * BPC:(k + 1) * BPC].reshape(BPC, C, PT)
            out.append(np.ascontiguousarray(
                sub.transpose(2, 0, 1).reshape(PT, BPC * C)))
        return out

    cols1_k = per_core_T(ecol1)
    cols2_k = per_core_T(ecol2)
    rid_k = per_core_T(erid)
    val_k = per_core_T(evalv)

    PPC = P2 // NCORES
    assert PPC % PT == 0
    PC = PPC // PT
    pp = perm[np.asarray(pairs).astype(np.int64)]
    psrc_k = [np.ascontiguousarray(
        pp[0, k * PPC:(k + 1) * PPC].reshape(PC, PT).T.astype(np.int32))
        for k in range(NCORES)]
    pdst_k = [np.ascontiguousarray(
        pp[1, k * PPC:(k + 1) * PPC].reshape(PC, PT).T.astype(np.int32))
        for k in range(NCORES)]

    types_pad = np.zeros(NPAD, np.int64)
    types_pad[:N] = types
    types_oh_t = np.zeros((TE, NPAD), F16)  # transposed one-hot, exact 0/1
    types_oh_t[types_pad, np.arange(NPAD)] = 1.0

    node_emb_pad = np.zeros((NPAD, H), F16)
    node_emb_pad[:N] = np.asarray(node_emb, np.float32).astype(F16)
    type_emb_pad = np.zeros((TE, H), F16)
    type_emb_pad[:T] = np.asarray(type_emb, np.float32).astype(F16)

    SH = BPC * PT  # x-shard rows per core
    Wp1 = np.asarray(Wp1, np.float32)
    shared = dict(
        type_emb=type_emb_pad,
        w1=np.asarray(W1, np.float32).astype(F16),
        w2=np.asarray(W2, np.float32).astype(F16),
        wp1a=np.ascontiguousarray(Wp1[0:D]).astype(F16),
        wp1b=np.ascontiguousarray(Wp1[D:2 * D]).astype(F16),
        wp1c=np.ascontiguousarray(Wp1[2 * D:3 * D]).astype(F16),
        wp2=np.asarray(Wp2, np.float32).astype(F16),
        b1bc=np.ascontiguousarray(np.broadcast_to(
            np.asarray(b1, np.float32), (PT, H))),
        b2bc=np.ascontiguousarray(np.broadcast_to(
            np.asarray(b2, np.float32), (PT, D))),
        bp1col=np.asarray(bp1, np.float32).reshape(D, 1),
        iota=np.ascontiguousarray(np.broadcast_to(
            np.arange(PT, dtype=np.float32), (PT, PT))).astype(F16),
        ident=np.eye(PT, dtype=F16),
    )
    per_core = [dict(cols1=cols1_k[k], cols2=cols2_k[k], ridt=rid_k[k],
                     valt=val_k[k], psrc=psrc_k[k], pdst=pdst_k[k],
                     node_emb=np.ascontiguousarray(
                         node_emb_pad[k * SH:(k + 1) * SH]),
                     types_oh=np.ascontiguousarray(
                         types_oh_t[:, k * SH:(k + 1) * SH]))
                for k in range(NCORES)]
    meta = dict(NPAD=NPAD, NBLK=NBLK, BPC=BPC, C=C, PC=PC, H=H, D=D,
                bp2f=float(np.asarray(bp2).reshape(-1)[0]))
    return meta, shared, per_core


def _build(meta):
    NPAD, NBLK, BPC, C, PC = (meta["NPAD"], meta["NBLK"], meta["BPC"],
                              meta["C"], meta["PC"])
    H, D, bp2f = meta["H"], meta["D"], meta["bp2f"]
    f32, f16, i32 = mybir.dt.float32, mybir.dt.float16, mybir.dt.int32
    AF = mybir.ActivationFunctionType
    Alu = mybir.AluOpType
    RG = [list(range(NCORES))]
    SH = BPC * PT

    nc = bacc.Bacc("TRN2", target_bir_lowering=False, debug=False,
                   num_devices=NCORES)

    # kernel I/O
    node_emb = nc.dram_tensor("node_emb", [SH, H], f16, kind="ExternalInput")
    type_emb = nc.dram_tensor("type_emb", [TE, H], f16, kind="ExternalInput")
    tyoh_d = nc.dram_tensor("types_oh", [TE, SH], f16, kind="ExternalInput")
    cols1_d = nc.dram_tensor("cols1", [PT, BPC * C], i32, kind="ExternalInput")
    cols2_d = nc.dram_tensor("cols2", [PT, BPC * C], i32, kind="ExternalInput")
    ridt_d = nc.dram_tensor("ridt", [PT, BPC * C], f32, kind="ExternalInput")
    valt_d = nc.dram_tensor("valt", [PT, BPC * C], f32, kind="ExternalInput")
    psrc_d = nc.dram_tensor("psrc", [PT, PC], i32, kind="ExternalInput")
    pdst_d = nc.dram_tensor("pdst", [PT, PC], i32, kind="ExternalInput")
    w1_d = nc.dram_tensor("w1", [H, H], f16, kind="ExternalInput")
    w2_d = nc.dram_tensor("w2", [H, D], f16, kind="ExternalInput")
    wp1a_d = nc.dram_tensor("wp1a", [D, D], f16, kind="ExternalInput")
    wp1b_d = nc.dram_tensor("wp1b", [D, D], f16, kind="ExternalInput")
    wp1c_d = nc.dram_tensor("wp1c", [D, D], f16, kind="ExternalInput")
    wp2_d = nc.dram_tensor("wp2", [D, 1], f16, kind="ExternalInput")
    b1bc_d = nc.dram_tensor("b1bc", [PT, H], f32, kind="ExternalInput")
    b2bc_d = nc.dram_tensor("b2bc", [PT, D], f32, kind="ExternalInput")
    bp1c_d = nc.dram_tensor("bp1col", [D, 1], f32, kind="ExternalInput")
    iota_d = nc.dram_tensor("iota", [PT, PT], f16, kind="ExternalInput")
    ident_d = nc.dram_tensor("ident", [PT, PT], f16, kind="ExternalInput")
    outp = nc.dram_tensor("out", [PC * PT, 1], f32, kind="ExternalOutput")

    # internal feature tables
    x_shard = nc.dram_tensor("x_shard", [SH, H], f16, kind="Internal")
    x_full = nc.dram_tensor("x_full", [NPAD, H], f16, kind="Internal",
                            addr_space="Shared")
    h_shard = nc.dram_tensor("h_shard", [SH, H], f16, kind="Internal")
    h_full = nc.dram_tensor("h_full", [NPAD, H], f16, kind="Internal",
                            addr_space="Shared")
    z_shard = nc.dram_tensor("z_shard", [SH, D], f16, kind="Internal")
    z_full = nc.dram_tensor("z_full", [NPAD, D], f16, kind="Internal",
                            addr_space="Shared")

    with tile.TileContext(nc) as tc:
        with (
            tc.tile_pool(name="const", bufs=1) as cpool,
            tc.tile_pool(name="idx", bufs=1) as ipool,
            tc.tile_pool(name="xne", bufs=3) as xnp,
            tc.tile_pool(name="gath", bufs=8) as gpool,
            tc.tile_pool(name="onep", bufs=8) as opool,
            tc.tile_pool(name="accs", bufs=3) as apool,
            tc.tile_pool(name="outs", bufs=3) as hpool,
            tc.tile_pool(name="pred", bufs=4) as ppool,
            tc.tile_pool(name="ps_agg", bufs=3, space="PSUM") as ps_agg,
            tc.tile_pool(name="ps_out", bufs=3, space="PSUM") as ps_out,
        ):
            def sb(pool, dram, shape, dtype):
                t = pool.tile(shape, dtype, name=dram.name + "_sb")
                nc.sync.dma_start(t[:], dram[:])
                return t

            # resident SBUF state
            iota_sb = sb(cpool, iota_d, [PT, PT], f16)
            ident_sb = sb(cpool, ident_d, [PT, PT], f16)
            w1_sb = sb(cpool, w1_d, [H, H], f16)
            w2_sb = sb(cpool, w2_d, [H, D], f16)
            wp1a_sb = sb(cpool, wp1a_d, [D, D], f16)
            wp1b_sb = sb(cpool, wp1b_d, [D, D], f16)
            wp1c_sb = sb(cpool, wp1c_d, [D, D], f16)
            wp2_sb = sb(cpool, wp2_d, [D, 1], f16)
            b1bc_sb = sb(cpool, b1bc_d, [PT, H], f32)
            b2bc_sb = sb(cpool, b2bc_d, [PT, D], f32)
            bp1c_sb = sb(cpool, bp1c_d, [D, 1], f32)
            tyemb_sb = sb(cpool, type_emb, [TE, H], f16)
            tyoh_sb = sb(ipool, tyoh_d, [TE, SH], f16)
            cols1_sb = sb(ipool, cols1_d, [PT, BPC * C], i32)
            cols2_sb = sb(ipool, cols2_d, [PT, BPC * C], i32)
            rid_sb = sb(ipool, ridt_d, [PT, BPC * C], f32)
            val_sb = sb(ipool, valt_d, [PT, BPC * C], f32)
            psrc_sb = sb(ipool, psrc_d, [PT, PC], i32)
            pdst_sb = sb(ipool, pdst_d, [PT, PC], i32)

            # ---- Phase X (sharded): x = node_emb + types_oh.T @ type_emb ----
            SB = 7 if BPC % 7 == 0 else 1  # blocks per supertile
            assert BPC % SB == 0
            for st in range(BPC // SB):
                a = st * SB * PT
                ne = xnp.tile([PT, SB * H], f16, name="ne")
                nc.sync.dma_start(
                    ne[:].rearrange("p (b f) -> p b f", b=SB),
                    node_emb[a:a + SB * PT, :].rearrange(
                        "(b p) f -> p b f", p=PT))
                xs = xnp.tile([PT, SB * H], f16, name="xs")
                for b in range(SB):
                    blk = st * SB + b
                    t_ps = ps_out.tile([PT, H], f32, name="o_ps", tag="o_ps")
                    nc.tensor.matmul(
                        t_ps[:], lhsT=tyoh_sb[:, blk * PT:(blk + 1) * PT],
                        rhs=tyemb_sb[:], start=True, stop=True)
                    nc.vector.tensor_add(
                        xs[:, b * H:(b + 1) * H], ne[:, b * H:(b + 1) * H],
                        t_ps[:])
                nc.sync.dma_start(
                    x_shard[a:a + SB * PT, :].rearrange("(b p) f -> p b f", p=PT),
                    xs[:].rearrange("p (b f) -> p b f", b=SB))
            nc.gpsimd.collective_compute(
                "AllGather", Alu.bypass, replica_groups=RG,
                ins=[x_shard[:]], outs=[x_full[:]])

            # ---- GCN layer ----
            def gcn_layer(src_table, colsb, dst_shard, w_sb, bias_sb, Dout,
                          relu, suffix):
                PAIR = 2 if C % 2 == 0 else 1
                for blk in range(BPC):
                    agg_ps = ps_agg.tile([PT, PT], f32, name="agg_ps")
                    for q in range(C // PAIR):
                        xg = gpool.tile([PT, PAIR * H], f16,
                                        name="xg" + suffix)
                        for j in range(PAIR):
                            g = blk * C + q * PAIR + j
                            nc.gpsimd.indirect_dma_start(
                                out=xg[:, j * H:(j + 1) * H], out_offset=None,
                                in_=src_table[:],
                                in_offset=bass.IndirectOffsetOnAxis(
                                    ap=colsb[:, g:g + 1], axis=0))
                        for j in range(PAIR):
                            c = q * PAIR + j
                            g = blk * C + c
                            oh = opool.tile([PT, PT], f16, name="oh" + suffix)
                            nc.vector.tensor_scalar(
                                oh[:], iota_sb[:], rid_sb[:, g:g + 1],
                                val_sb[:, g:g + 1], op0=Alu.is_equal,
                                op1=Alu.mult)
                            nc.tensor.matmul(
                                agg_ps[:], lhsT=xg[:, j * H:(j + 1) * H],
                                rhs=oh[:], start=(c == 0), stop=(c == C - 1))
                    aggT_sb = apool.tile([PT, PT], f16, name="aggT" + suffix)
                    nc.vector.tensor_copy(aggT_sb[:], agg_ps[:])
                    o_ps = ps_out.tile([PT, Dout], f32, name="o_ps", tag="o_ps")
                    nc.tensor.matmul(o_ps[:], lhsT=aggT_sb[:], rhs=w_sb[:],
                                     start=True, stop=True)
                    o_sb = hpool.tile([PT, Dout], f16, name="osb" + suffix)
                    if relu:
                        ob = hpool.tile([PT, Dout], f32, name="ob" + suffix)
                        nc.vector.tensor_add(ob[:], o_ps[:], bias_sb[:])
                        nc.scalar.activation(o_sb[:], ob[:], AF.Relu)
                    else:
                        nc.vector.tensor_add(o_sb[:], o_ps[:], bias_sb[:])
                    nc.sync.dma_start(
                        dst_shard[blk * PT:(blk + 1) * PT, :], o_sb[:])

            gcn_layer(x_full, cols1_sb, h_shard, w1_sb, b1bc_sb, H,
                      relu=True, suffix="1")
            nc.gpsimd.collective_compute(
                "AllGather", Alu.bypass, replica_groups=RG,
                ins=[h_shard[:]], outs=[h_full[:]])

            gcn_layer(h_full, cols2_sb, z_shard, w2_sb, b2bc_sb, D,
                      relu=False, suffix="2")
            nc.gpsimd.collective_compute(
                "AllGather", Alu.bypass, replica_groups=RG,
                ins=[z_shard[:]], outs=[z_full[:]])

            # ---- predictor ----
            for pc in range(PC):
                sg = ppool.tile([PT, D], f16, name="sg")
                nc.gpsimd.indirect_dma_start(
                    out=sg[:], out_offset=None, in_=z_full[:],
                    in_offset=bass.IndirectOffsetOnAxis(
                        ap=psrc_sb[:, pc:pc + 1], axis=0))
                dg = ppool.tile([PT, D], f16, name="dg")
                nc.gpsimd.indirect_dma_start(
                    out=dg[:], out_offset=None, in_=z_full[:],
                    in_offset=bass.IndirectOffsetOnAxis(
                        ap=pdst_sb[:, pc:pc + 1], axis=0))
                sgt_ps = ps_out.tile([D, PT], f16, name="sgt_ps", tag="o_ps")
                nc.tensor.transpose(sgt_ps[:], sg[:], ident_sb[:])
                dgt_ps = ps_out.tile([D, PT], f16, name="dgt_ps", tag="o_ps")
                nc.tensor.transpose(dgt_ps[:], dg[:], ident_sb[:])
                sgt = ppool.tile([D, PT], f16, name="sgt")
                nc.vector.tensor_copy(sgt[:], sgt_ps[:])
                dgt = ppool.tile([D, PT], f16, name="dgt")
                nc.vector.tensor_copy(dgt[:], dgt_ps[:])
                sdt = ppool.tile([D, PT], f16, name="sdt")
                nc.vector.tensor_mul(sdt[:], sgt[:], dgt[:])
                yt_ps = ps_agg.tile([D, PT], f32, name="yt_ps", tag="agg_ps")
                nc.tensor.matmul(yt_ps[:], lhsT=wp1a_sb[:], rhs=sgt[:],
                                 start=True, stop=False)
                nc.tensor.matmul(yt_ps[:], lhsT=wp1b_sb[:], rhs=dgt[:],
                                 start=False, stop=False)
                nc.tensor.matmul(yt_ps[:], lhsT=wp1c_sb[:], rhs=sdt[:],
                                 start=False, stop=True)
                r_sb = ppool.tile([D, PT], f16, name="r_sb")
                nc.scalar.activation(r_sb[:], yt_ps[:], AF.Relu,
                                     bias=bp1c_sb[:])
                o2_ps = ps_out.tile([PT, 1], f32, name="o2_ps", tag="o_ps")
                nc.tensor.matmul(o2_ps[:], lhsT=r_sb[:], rhs=wp2_sb[:],
                                 start=True, stop=True)
                o2_sb = ppool.tile([PT, 1], f32, name="o2_sb")
                nc.scalar.activation(o2_sb[:], o2_ps[:], AF.Copy, bias=bp2f)
                nc.sync.dma_start(outp[pc * PT:(pc + 1) * PT, :], o2_sb[:])

    nc.compile()
    return nc


def kernel(**inputs) -> np.ndarray:
    meta, shared, per_core = _preprocess(**inputs)
    key = tuple(sorted(meta.items()))
    if key not in _prog_cache:
        _prog_cache[key] = _build(meta)
    nc = _prog_cache[key]
    in_maps = [dict(shared, **per_core[k]) for k in range(NCORES)]
    res = bass_utils.run_bass_kernel_spmd(
        nc, in_maps, core_ids=list(range(NCORES)))
    out = np.concatenate(
        [np.asarray(res.results[k]["out"])[:, 0] for k in range(NCORES)])
    return out.astype(np.float32)


# revision 11
# speedup vs baseline: 1.0030x; 1.0030x over previous
"""Trainium2 Bass kernel for a 2-layer GCN + link predictor (PrimeKG drug
repurposing GNN).

Strategy (8 NeuronCores, SPMD single program):
  - Nodes are permuted into 128-node "buckets" balanced by in-degree; each
    core owns NBLK/8 consecutive buckets (rows of the aggregation).
  - Edges are grouped by destination bucket and padded to chunks of 128.
    segment_sum is computed per bucket as a sequence of PE matmuls:
       aggT[f, n] += sum_e xgath[e, f] * onehot[e, n]
    where xgath is an indirect-DMA gather of source-node features and
    onehot[e, n] = vals[e] * (n == local_row[e]) built on the vector engine.
  - x = node_emb + type_onehot.T @ type_emb is built sharded (original node
    order, host-precomputed transposed one-hot) and AllGathered; h and z
    live in permuted order, sharded and AllGathered likewise.
  - Pairs are sharded by batch; the predictor gathers z rows, transposes on
    the PE, and runs the tiny MLP per 128-pair chunk.

Feature tables and matmuls are fp16 with f32 PSUM accumulation.
"""

import numpy as np

import concourse.bass as bass
import concourse.bacc as bacc
import concourse.tile as tile
import concourse.mybir as mybir
from concourse import bass_utils

PT = 128  # partitions
NCORES = 8
TE = 16   # padded type-embedding rows

F16 = np.float16

_prog_cache: dict = {}


def _preprocess(node_type_ids, adj_rows, adj_cols, adj_vals, pairs,
                node_emb, type_emb, W1, b1, W2, b2, Wp1, bp1, Wp2, bp2):
    N, H = node_emb.shape
    T = type_emb.shape[0]
    E = adj_rows.shape[0]
    P2 = pairs.shape[1]
    D = W2.shape[1]
    assert H == PT and T <= TE and Wp1.shape == (3 * D, D)

    NPB = PT * NCORES
    NPAD = -(-N // NPB) * NPB
    NBLK = NPAD // PT
    BPC = NBLK // NCORES

    rows = np.asarray(adj_rows).astype(np.int64)
    cols = np.asarray(adj_cols).astype(np.int64)
    vals = np.asarray(adj_vals).astype(np.float32)
    types = np.asarray(node_type_ids).astype(np.int64)

    # Degree-balanced bucket assignment: deal nodes (sorted by in-degree
    # desc) round-robin across the NBLK buckets, then repair-swap nodes
    # between heavy and light buckets to pull the max bucket load down to
    # the next multiple-of-128 boundary.
    deg = np.bincount(rows, minlength=N).astype(np.int64)
    deg_pad = np.zeros(NPAD, np.int64)
    deg_pad[:N] = deg
    order = np.argsort(-deg_pad, kind="stable")
    i = np.arange(NPAD)
    bucket_of_rank = i % NBLK
    slot_of_rank = i // NBLK
    bucket_of = np.empty(NPAD, np.int64)
    bucket_of[order] = bucket_of_rank
    loads = np.bincount(bucket_of, weights=deg_pad, minlength=NBLK).astype(
        np.int64)
    target = max(PT, int(-(-int(loads.max()) // PT) - 1) * PT)
    members = [list(order[b::NBLK][::-1]) for b in range(NBLK)]  # asc degree
    for _ in range(2000):
        hb = int(np.argmax(loads))
        if loads[hb] <= target:
            break
        lb = int(np.argmin(loads))
        done = False
        for mi in range(len(members[hb]) - 1, -1, -1):
            m = members[hb][mi]
            for li, l in enumerate(members[lb]):
                delta = deg_pad[m] - deg_pad[l]
                if delta <= 0:
                    break
                if loads[lb] + delta <= target:
                    members[hb][mi], members[lb][li] = l, m
                    loads[hb] -= delta
                    loads[lb] += delta
                    done = True
                    break
            if done:
                break
        if not done:
            break
    perm = np.empty(N, np.int64)
    for b in range(NBLK):
        for s, m in enumerate(members[b]):
            if m < N:
                perm[m] = b * PT + s

    prow = perm[rows]
    bkt = prow // PT
    rid = (prow % PT).astype(np.float32)
    cnt = np.bincount(bkt, minlength=NBLK)
    C = max(1, int(-(-int(cnt.max()) // PT)))
    CAP = C * PT

    eord = np.argsort(bkt, kind="stable")
    bs = bkt[eord]
    starts = np.concatenate([[0], np.cumsum(cnt)[:-1]])
    ps = np.arange(E) - starts[bs]

    ecol1 = np.zeros((NBLK, CAP), np.int32)
    ecol2 = np.zeros((NBLK, CAP), np.int32)
    erid = np.zeros((NBLK, CAP), np.float32)
    evalv = np.zeros((NBLK, CAP), np.float32)
    ce = cols[eord]
    ecol1[bs, ps] = ce
    ecol2[bs, ps] = perm[ce]
    erid[bs, ps] = rid[eord]
    evalv[bs, ps] = vals[eord]

    def per_core_T(a):
        # [NBLK, C*PT] -> per-core [PT, BPC*C]; column blk*C+c, partition p
        # holds bucket (core*BPC+blk) edge slot c*128+p.
        out = []
        for k in range(NCORES):
            sub = a[k * BPC:(k + 1) * BPC].reshape(BPC, C, PT)
            out.append(np.ascontiguousarray(
                sub.transpose(2, 0, 1).reshape(PT, BPC * C)))
        return out

    cols1_k = per_core_T(ecol1)
    cols2_k = per_core_T(ecol2)
    rid_k = per_core_T(erid)
    val_k = per_core_T(evalv)

    PPC = P2 // NCORES
    assert PPC % PT == 0
    PC = PPC // PT
    pp = perm[np.asarray(pairs).astype(np.int64)]
    psrc_k = [np.ascontiguousarray(
        pp[0, k * PPC:(k + 1) * PPC].reshape(PC, PT).T.astype(np.int32))
        for k in range(NCORES)]
    pdst_k = [np.ascontiguousarray(
        pp[1, k * PPC:(k + 1) * PPC].reshape(PC, PT).T.astype(np.int32))
        for k in range(NCORES)]

    types_pad = np.zeros(NPAD, np.int64)
    types_pad[:N] = types
    types_oh_t = np.zeros((TE, NPAD), F16)  # transposed one-hot, exact 0/1
    types_oh_t[types_pad, np.arange(NPAD)] = 1.0

    node_emb_pad = np.zeros((NPAD, H), F16)
    node_emb_pad[:N] = np.asarray(node_emb, np.float32).astype(F16)
    type_emb_pad = np.zeros((TE, H), F16)
    type_emb_pad[:T] = np.asarray(type_emb, np.float32).astype(F16)

    SH = BPC * PT  # x-shard rows per core
    Wp1 = np.asarray(Wp1, np.float32)
    shared = dict(
        type_emb=type_emb_pad,
        w1=np.asarray(W1, np.float32).astype(F16),
        w2=np.asarray(W2, np.float32).astype(F16),
        wp1a=np.ascontiguousarray(Wp1[0:D]).astype(F16),
        wp1b=np.ascontiguousarray(Wp1[D:2 * D]).astype(F16),
        wp1c=np.ascontiguousarray(Wp1[2 * D:3 * D]).astype(F16),
        wp2=np.asarray(Wp2, np.float32).astype(F16),
        b1bc=np.ascontiguousarray(np.broadcast_to(
            np.asarray(b1, np.float32), (PT, H))),
        b2bc=np.ascontiguousarray(np.broadcast_to(
            np.asarray(b2, np.float32), (PT, D))),
        bp1col=np.asarray(bp1, np.float32).reshape(D, 1),
        iota=np.ascontiguousarray(np.broadcast_to(
            np.arange(PT, dtype=np.float32), (PT, PT))).astype(F16),
        ident=np.eye(PT, dtype=F16),
    )
    per_core = [dict(cols1=cols1_k[k], cols2=cols2_k[k], ridt=rid_k[k],
                     valt=val_k[k], psrc=psrc_k[k], pdst=pdst_k[k],
                     node_emb=np.ascontiguousarray(
                         node_emb_pad[k * SH:(k + 1) * SH]),
                     types_oh=np.ascontiguousarray(
                         types_oh_t[:, k * SH:(k + 1) * SH]))
                for k in range(NCORES)]
    meta = dict(NPAD=NPAD, NBLK=NBLK, BPC=BPC, C=C, PC=PC, H=H, D=D,
                bp2f=float(np.asarray(bp2).reshape(-1)[0]))
    return meta, shared, per_core


def _build(meta):
    NPAD, NBLK, BPC, C, PC = (meta["NPAD"], meta["NBLK"], meta["BPC"],
                              meta["C"], meta["PC"])
    H, D, bp2f = meta["H"], meta["D"], meta["bp2f"]
    f32, f16, i32 = mybir.dt.float32, mybir.dt.float16, mybir.dt.int32
    AF = mybir.ActivationFunctionType
    Alu = mybir.AluOpType
    RG = [list(range(NCORES))]
    SH = BPC * PT

    nc = bacc.Bacc("TRN2", target_bir_lowering=False, debug=False,
                   num_devices=NCORES)

    # kernel I/O
    node_emb = nc.dram_tensor("node_emb", [SH, H], f16, kind="ExternalInput")
    type_emb = nc.dram_tensor("type_emb", [TE, H], f16, kind="ExternalInput")
    tyoh_d = nc.dram_tensor("types_oh", [TE, SH], f16, kind="ExternalInput")
    cols1_d = nc.dram_tensor("cols1", [PT, BPC * C], i32, kind="ExternalInput")
    cols2_d = nc.dram_tensor("cols2", [PT, BPC * C], i32, kind="ExternalInput")
    ridt_d = nc.dram_tensor("ridt", [PT, BPC * C], f32, kind="ExternalInput")
    valt_d = nc.dram_tensor("valt", [PT, BPC * C], f32, kind="ExternalInput")
    psrc_d = nc.dram_tensor("psrc", [PT, PC], i32, kind="ExternalInput")
    pdst_d = nc.dram_tensor("pdst", [PT, PC], i32, kind="ExternalInput")
    w1_d = nc.dram_tensor("w1", [H, H], f16, kind="ExternalInput")
    w2_d = nc.dram_tensor("w2", [H, D], f16, kind="ExternalInput")
    wp1a_d = nc.dram_tensor("wp1a", [D, D], f16, kind="ExternalInput")
    wp1b_d = nc.dram_tensor("wp1b", [D, D], f16, kind="ExternalInput")
    wp1c_d = nc.dram_tensor("wp1c", [D, D], f16, kind="ExternalInput")
    wp2_d = nc.dram_tensor("wp2", [D, 1], f16, kind="ExternalInput")
    b1bc_d = nc.dram_tensor("b1bc", [PT, H], f32, kind="ExternalInput")
    b2bc_d = nc.dram_tensor("b2bc", [PT, D], f32, kind="ExternalInput")
    bp1c_d = nc.dram_tensor("bp1col", [D, 1], f32, kind="ExternalInput")
    iota_d = nc.dram_tensor("iota", [PT, PT], f16, kind="ExternalInput")
    ident_d = nc.dram_tensor("ident", [PT, PT], f16, kind="ExternalInput")
    outp = nc.dram_tensor("out", [PC * PT, 1], f32, kind="ExternalOutput")

    # internal feature tables
    x_shard = nc.dram_tensor("x_shard", [SH, H], f16, kind="Internal")
    x_full = nc.dram_tensor("x_full", [NPAD, H], f16, kind="Internal",
                            addr_space="Shared")
    h_shard = nc.dram_tensor("h_shard", [SH, H], f16, kind="Internal")
    h_full = nc.dram_tensor("h_full", [NPAD, H], f16, kind="Internal",
                            addr_space="Shared")
    z_shard = nc.dram_tensor("z_shard", [SH, D], f16, kind="Internal")
    z_full = nc.dram_tensor("z_full", [NPAD, D], f16, kind="Internal",
                            addr_space="Shared")

    with tile.TileContext(nc) as tc:
        with (
            tc.tile_pool(name="const", bufs=1) as cpool,
            tc.tile_pool(name="idx", bufs=1) as ipool,
            tc.tile_pool(name="xne", bufs=3) as xnp,
            tc.tile_pool(name="gath", bufs=8) as gpool,
            tc.tile_pool(name="onep", bufs=8) as opool,
            tc.tile_pool(name="accs", bufs=3) as apool,
            tc.tile_pool(name="outs", bufs=3) as hpool,
            tc.tile_pool(name="pred", bufs=4) as ppool,
            tc.tile_pool(name="ps_agg", bufs=3, space="PSUM") as ps_agg,
            tc.tile_pool(name="ps_out", bufs=3, space="PSUM") as ps_out,
        ):
            def sb(pool, dram, shape, dtype):
                t = pool.tile(shape, dtype, name=dram.name + "_sb")
                nc.sync.dma_start(t[:], dram[:])
                return t

            # resident SBUF state
            iota_sb = sb(cpool, iota_d, [PT, PT], f16)
            ident_sb = sb(cpool, ident_d, [PT, PT], f16)
            w1_sb = sb(cpool, w1_d, [H, H], f16)
            w2_sb = sb(cpool, w2_d, [H, D], f16)
            wp1a_sb = sb(cpool, wp1a_d, [D, D], f16)
            wp1b_sb = sb(cpool, wp1b_d, [D, D], f16)
            wp1c_sb = sb(cpool, wp1c_d, [D, D], f16)
            wp2_sb = sb(cpool, wp2_d, [D, 1], f16)
            b1bc_sb = sb(cpool, b1bc_d, [PT, H], f32)
            b2bc_sb = sb(cpool, b2bc_d, [PT, D], f32)
            bp1c_sb = sb(cpool, bp1c_d, [D, 1], f32)
            tyemb_sb = sb(cpool, type_emb, [TE, H], f16)
            tyoh_sb = sb(ipool, tyoh_d, [TE, SH], f16)
            cols1_sb = sb(ipool, cols1_d, [PT, BPC * C], i32)
            cols2_sb = sb(ipool, cols2_d, [PT, BPC * C], i32)
            rid_sb = sb(ipool, ridt_d, [PT, BPC * C], f32)
            val_sb = sb(ipool, valt_d, [PT, BPC * C], f32)
            psrc_sb = sb(ipool, psrc_d, [PT, PC], i32)
            pdst_sb = sb(ipool, pdst_d, [PT, PC], i32)

            # ---- Phase X (sharded): x = node_emb + types_oh.T @ type_emb ----
            SB = 7 if BPC % 7 == 0 else 1  # blocks per supertile
            assert BPC % SB == 0
            for st in range(BPC // SB):
                a = st * SB * PT
                ne = xnp.tile([PT, SB * H], f16, name="ne")
                nc.sync.dma_start(
                    ne[:].rearrange("p (b f) -> p b f", b=SB),
                    node_emb[a:a + SB * PT, :].rearrange(
                        "(b p) f -> p b f", p=PT))
                xs = xnp.tile([PT, SB * H], f16, name="xs")
                for b in range(SB):
                    blk = st * SB + b
                    t_ps = ps_out.tile([PT, H], f32, name="o_ps", tag="o_ps")
                    nc.tensor.matmul(
                        t_ps[:], lhsT=tyoh_sb[:, blk * PT:(blk + 1) * PT],
                        rhs=tyemb_sb[:], start=True, stop=True)
                    nc.vector.tensor_add(
                        xs[:, b * H:(b + 1) * H], ne[:, b * H:(b + 1) * H],
                        t_ps[:])
                nc.sync.dma_start(
                    x_shard[a:a + SB * PT, :].rearrange("(b p) f -> p b f", p=PT),
                    xs[:].rearrange("p (b f) -> p b f", b=SB))
            nc.gpsimd.collective_compute(
                "AllGather", Alu.bypass, replica_groups=RG,
                ins=[x_shard[:]], outs=[x_full[:]])

            # ---- GCN layer ----
            def gcn_layer(src_table, colsb, dst_shard, w_sb, bias_sb, Dout,
                          relu, suffix):
                PAIR = 1
                for blk in range(BPC):
                    agg_ps = ps_agg.tile([PT, PT], f32, name="agg_ps")
                    for q in range(C // PAIR):
                        xg = gpool.tile([PT, PAIR * H], f16,
                                        name="xg" + suffix)
                        for j in range(PAIR):
                            g = blk * C + q * PAIR + j
                            nc.gpsimd.indirect_dma_start(
                                out=xg[:, j * H:(j + 1) * H], out_offset=None,
                                in_=src_table[:],
                                in_offset=bass.IndirectOffsetOnAxis(
                                    ap=colsb[:, g:g + 1], axis=0))
                        for j in range(PAIR):
                            c = q * PAIR + j
                            g = blk * C + c
                            oh = opool.tile([PT, PT], f16, name="oh" + suffix)
                            nc.vector.tensor_scalar(
                                oh[:], iota_sb[:], rid_sb[:, g:g + 1],
                                val_sb[:, g:g + 1], op0=Alu.is_equal,
                                op1=Alu.mult)
                            nc.tensor.matmul(
                                agg_ps[:], lhsT=xg[:, j * H:(j + 1) * H],
                                rhs=oh[:], start=(c == 0), stop=(c == C - 1))
                    aggT_sb = apool.tile([PT, PT], f16, name="aggT" + suffix)
                    nc.vector.tensor_copy(aggT_sb[:], agg_ps[:])
                    o_ps = ps_out.tile([PT, Dout], f32, name="o_ps", tag="o_ps")
                    nc.tensor.matmul(o_ps[:], lhsT=aggT_sb[:], rhs=w_sb[:],
                                     start=True, stop=True)
                    o_sb = hpool.tile([PT, Dout], f16, name="osb" + suffix)
                    if relu:
                        ob = hpool.tile([PT, Dout], f32, name="ob" + suffix)
                        nc.vector.tensor_add(ob[:], o_ps[:], bias_sb[:])
                        nc.scalar.activation(o_sb[:], ob[:], AF.Relu)
                    else:
                        nc.vector.tensor_add(o_sb[:], o_ps[:], bias_sb[:])
                    nc.sync.dma_start(
                        dst_shard[blk * PT:(blk + 1) * PT, :], o_sb[:])

            gcn_layer(x_full, cols1_sb, h_shard, w1_sb, b1bc_sb, H,
                      relu=True, suffix="1")
            nc.gpsimd.collective_compute(
                "AllGather", Alu.bypass, replica_groups=RG,
                ins=[h_shard[:]], outs=[h_full[:]])

            gcn_layer(h_full, cols2_sb, z_shard, w2_sb, b2bc_sb, D,
                      relu=False, suffix="2")
            nc.gpsimd.collective_compute(
                "AllGather", Alu.bypass, replica_groups=RG,
                ins=[z_shard[:]], outs=[z_full[:]])

            # ---- predictor ----
            for pc in range(PC):
                sg = ppool.tile([PT, D], f16, name="sg")
                nc.gpsimd.indirect_dma_start(
                    out=sg[:], out_offset=None, in_=z_full[:],
                    in_offset=bass.IndirectOffsetOnAxis(
                        ap=psrc_sb[:, pc:pc + 1], axis=0))
                dg = ppool.tile([PT, D], f16, name="dg")
                nc.gpsimd.indirect_dma_start(
                    out=dg[:], out_offset=None, in_=z_full[:],
                    in_offset=bass.IndirectOffsetOnAxis(
                        ap=pdst_sb[:, pc:pc + 1], axis=0))
                sgt_ps = ps_out.tile([D, PT], f16, name="sgt_ps", tag="o_ps")
                nc.tensor.transpose(sgt_ps[:], sg[:], ident_sb[:])
                dgt_ps = ps_out.tile([D, PT], f16, name="dgt_ps", tag="o_ps")
                nc.tensor.transpose(dgt_ps[:], dg[:], ident_sb[:])
                sgt = ppool.tile([D, PT], f16, name="sgt")
                nc.vector.tensor_copy(sgt[:], sgt_ps[:])
                dgt = ppool.tile([D, PT], f16, name="dgt")
                nc.vector.tensor_copy(dgt[:], dgt_ps[:])
                sdt = ppool.tile([D, PT], f16, name="sdt")
                nc.vector.tensor_mul(sdt[:], sgt[:], dgt[:])
                yt_ps = ps_agg.tile([D, PT], f32, name="yt_ps", tag="agg_ps")
                nc.tensor.matmul(yt_ps[:], lhsT=wp1a_sb[:], rhs=sgt[:],
                                 start=True, stop=False)
                nc.tensor.matmul(yt_ps[:], lhsT=wp1b_sb[:], rhs=dgt[:],
                                 start=False, stop=False)
                nc.tensor.matmul(yt_ps[:], lhsT=wp1c_sb[:], rhs=sdt[:],
                                 start=False, stop=True)
                r_sb = ppool.tile([D, PT], f16, name="r_sb")
                nc.scalar.activation(r_sb[:], yt_ps[:], AF.Relu,
                                     bias=bp1c_sb[:])
                o2_ps = ps_out.tile([PT, 1], f32, name="o2_ps", tag="o_ps")
                nc.tensor.matmul(o2_ps[:], lhsT=r_sb[:], rhs=wp2_sb[:],
                                 start=True, stop=True)
                o2_sb = ppool.tile([PT, 1], f32, name="o2_sb")
                nc.scalar.activation(o2_sb[:], o2_ps[:], AF.Copy, bias=bp2f)
                nc.sync.dma_start(outp[pc * PT:(pc + 1) * PT, :], o2_sb[:])

    nc.compile()
    return nc


def kernel(**inputs) -> np.ndarray:
    meta, shared, per_core = _preprocess(**inputs)
    key = tuple(sorted(meta.items()))
    if key not in _prog_cache:
        _prog_cache[key] = _build(meta)
    nc = _prog_cache[key]
    in_maps = [dict(shared, **per_core[k]) for k in range(NCORES)]
    res = bass_utils.run_bass_kernel_spmd(
        nc, in_maps, core_ids=list(range(NCORES)))
    out = np.concatenate(
        [np.asarray(res.results[k]["out"])[:, 0] for k in range(NCORES)])
    return out.astype(np.float32)


# revision 19
# speedup vs baseline: 1.0393x; 1.0362x over previous
"""Trainium2 Bass kernel for a 2-layer GCN + link predictor (PrimeKG drug
repurposing GNN).

Strategy (8 NeuronCores, SPMD single program):
  - Nodes are permuted into 128-node "buckets" balanced by in-degree; each
    core owns NBLK/8 consecutive buckets (rows of the aggregation).
  - Edges are grouped by destination bucket and padded to chunks of 128.
    segment_sum is computed per bucket as a sequence of PE matmuls:
       aggT[f, n] += sum_e xgath[e, f] * onehot[e, n]
    where xgath is an indirect-DMA gather of source-node features and
    onehot[e, n] = vals[e] * (n == local_row[e]) built on the vector engine.
  - x = node_emb + type_onehot.T @ type_emb is built sharded (original node
    order, host-precomputed transposed one-hot) and AllGathered; h and z
    live in permuted order, sharded and AllGathered likewise.
  - Pairs are sharded by batch; the predictor gathers z rows, transposes on
    the PE, and runs the tiny MLP per 128-pair chunk.

Feature tables and matmuls are fp16 with f32 PSUM accumulation.
"""

import numpy as np

import concourse.bass as bass
import concourse.bacc as bacc
import concourse.tile as tile
import concourse.mybir as mybir
from concourse import bass_utils

PT = 128  # partitions
NCORES = 8
TE = 16   # padded type-embedding rows

F16 = np.float16

_prog_cache: dict = {}


def _preprocess(node_type_ids, adj_rows, adj_cols, adj_vals, pairs,
                node_emb, type_emb, W1, b1, W2, b2, Wp1, bp1, Wp2, bp2):
    N, H = node_emb.shape
    T = type_emb.shape[0]
    E = adj_rows.shape[0]
    P2 = pairs.shape[1]
    D = W2.shape[1]
    assert H == PT and T <= TE and Wp1.shape == (3 * D, D)

    NPB = PT * NCORES
    NPAD = -(-N // NPB) * NPB
    NBLK = NPAD // PT
    BPC = NBLK // NCORES

    rows = np.asarray(adj_rows).astype(np.int64)
    cols = np.asarray(adj_cols).astype(np.int64)
    vals = np.asarray(adj_vals).astype(np.float32)
    types = np.asarray(node_type_ids).astype(np.int64)

    # Degree-balanced bucket assignment: deal nodes (sorted by in-degree
    # desc) round-robin across the NBLK buckets, then repair-swap nodes
    # between heavy and light buckets to pull the max bucket load down to
    # the next multiple-of-128 boundary.
    deg = np.bincount(rows, minlength=N).astype(np.int64)
    deg_pad = np.zeros(NPAD, np.int64)
    deg_pad[:N] = deg
    order = np.argsort(-deg_pad, kind="stable")
    i = np.arange(NPAD)
    bucket_of_rank = i % NBLK
    slot_of_rank = i // NBLK
    bucket_of = np.empty(NPAD, np.int64)
    bucket_of[order] = bucket_of_rank
    loads = np.bincount(bucket_of, weights=deg_pad, minlength=NBLK).astype(
        np.int64)
    target = max(PT, int(-(-int(loads.max()) // PT) - 1) * PT)
    members = [list(order[b::NBLK][::-1]) for b in range(NBLK)]  # asc degree
    for _ in range(2000):
        hb = int(np.argmax(loads))
        if loads[hb] <= target:
            break
        lb = int(np.argmin(loads))
        done = False
        for mi in range(len(members[hb]) - 1, -1, -1):
            m = members[hb][mi]
            for li, l in enumerate(members[lb]):
                delta = deg_pad[m] - deg_pad[l]
                if delta <= 0:
                    break
                if loads[lb] + delta <= target:
                    members[hb][mi], members[lb][li] = l, m
                    loads[hb] -= delta
                    loads[lb] += delta
                    done = True
                    break
            if done:
                break
        if not done:
            break
    perm = np.empty(N, np.int64)
    for b in range(NBLK):
        for s, m in enumerate(members[b]):
            if m < N:
                perm[m] = b * PT + s

    prow = perm[rows]
    bkt = prow // PT
    rid = (prow % PT).astype(np.float32)
    cnt = np.bincount(bkt, minlength=NBLK)
    SH = BPC * PT

    # Per-layer edge layout, local-first: chunk 0 of each bucket holds only
    # edges whose gather source row lives in the processing core's shard
    # (gathered from the shard table before the AllGather completes);
    # remaining edges go to chunks 1.. with full-table ids.
    def layer_arrays(tid):
        owner = bkt // BPC
        local = (tid // SH) == owner
        eo = np.lexsort((~local, bkt))
        b2 = bkt[eo]
        l2 = local[eo]
        t2 = tid[eo]
        r2 = rid[eo]
        v2 = vals[eo]
        cnts = np.bincount(b2, minlength=NBLK)
        st = np.concatenate([[0], np.cumsum(cnts)[:-1]])
        pos = np.arange(E) - st[b2]
        nl = np.bincount(b2, weights=l2.astype(np.float64),
                         minlength=NBLK).astype(np.int64)
        nl0 = np.minimum(nl, PT)
        slot = np.where(l2 & (pos < PT), pos, PT + pos - nl0[b2])
        owner2 = b2 // BPC
        ids = np.where(slot < PT, t2 - owner2 * SH, t2).astype(np.int32)
        Cl = int(-(-int((PT + cnts - nl0).max()) // PT))
        return b2, slot, ids, r2, v2, Cl

    b2a, slot_a, ids_a, rid_a, val_a, C1 = layer_arrays(cols)
    b2b, slot_b, ids_b, rid_b, val_b, C2 = layer_arrays(perm[cols])
    C = max(C1, C2)
    CAP = C * PT

    def fill(b2, slot, ids, r2, v2):
        ec = np.zeros((NBLK, CAP), np.int32)
        er = np.zeros((NBLK, CAP), np.float32)
        ev = np.zeros((NBLK, CAP), np.float32)
        ec[b2, slot] = ids
        er[b2, slot] = r2
        ev[b2, slot] = v2
        return ec, er, ev

    ecol1, erid1, eval1 = fill(b2a, slot_a, ids_a, rid_a, val_a)
    ecol2, erid2, eval2 = fill(b2b, slot_b, ids_b, rid_b, val_b)

    def per_core_T(a):
        # [NBLK, C*PT] -> per-core [PT, BPC*C]; column blk*C+c, partition p
        # holds bucket (core*BPC+blk) edge slot c*128+p.
        out = []
        for k in range(NCORES):
            sub = a[k * BPC:(k + 1) * BPC].reshape(BPC, C, PT)
            out.append(np.ascontiguousarray(
                sub.transpose(2, 0, 1).reshape(PT, BPC * C)))
        return out

    cols1_k = per_core_T(ecol1)
    rid1_k = per_core_T(erid1)
    val1_k = per_core_T(eval1)
    cols2_k = per_core_T(ecol2)
    rid2_k = per_core_T(erid2)
    val2_k = per_core_T(eval2)

    PPC = P2 // NCORES
    assert PPC % PT == 0
    PC = PPC // PT
    pp = perm[np.asarray(pairs).astype(np.int64)]
    psrc_k = [np.ascontiguousarray(
        pp[0, k * PPC:(k + 1) * PPC].reshape(PC, PT).T.astype(np.int32))
        for k in range(NCORES)]
    pdst_k = [np.ascontiguousarray(
        pp[1, k * PPC:(k + 1) * PPC].reshape(PC, PT).T.astype(np.int32))
        for k in range(NCORES)]

    types_pad = np.zeros(NPAD, np.int64)
    types_pad[:N] = types
    types_oh_t = np.zeros((TE, NPAD), F16)  # transposed one-hot, exact 0/1
    types_oh_t[types_pad, np.arange(NPAD)] = 1.0

    node_emb_pad = np.zeros((NPAD, H), F16)
    node_emb_pad[:N] = np.asarray(node_emb, np.float32).astype(F16)
    type_emb_pad = np.zeros((TE, H), F16)
    type_emb_pad[:T] = np.asarray(type_emb, np.float32).astype(F16)

    SH = BPC * PT  # x-shard rows per core
    Wp1 = np.asarray(Wp1, np.float32)
    shared = dict(
        type_emb=type_emb_pad,
        w1=np.asarray(W1, np.float32).astype(F16),
        w2=np.asarray(W2, np.float32).astype(F16),
        wp1a=np.ascontiguousarray(Wp1[0:D]).astype(F16),
        wp1b=np.ascontiguousarray(Wp1[D:2 * D]).astype(F16),
        wp1c=np.ascontiguousarray(Wp1[2 * D:3 * D]).astype(F16),
        wp2=np.asarray(Wp2, np.float32).astype(F16),
        b1bc=np.ascontiguousarray(np.broadcast_to(
            np.asarray(b1, np.float32), (PT, H))),
        b2bc=np.ascontiguousarray(np.broadcast_to(
            np.asarray(b2, np.float32), (PT, D))),
        bp1col=np.asarray(bp1, np.float32).reshape(D, 1),
        iota=np.ascontiguousarray(np.broadcast_to(
            np.arange(PT, dtype=np.float32), (PT, PT))).astype(F16),
        ident=np.eye(PT, dtype=F16),
    )
    per_core = [dict(cols1=cols1_k[k], cols2=cols2_k[k], rid1=rid1_k[k],
                     val1=val1_k[k], rid2=rid2_k[k], val2=val2_k[k],
                     psrc=psrc_k[k], pdst=pdst_k[k],
                     node_emb=np.ascontiguousarray(
                         node_emb_pad[k * SH:(k + 1) * SH]),
                     types_oh=np.ascontiguousarray(
                         types_oh_t[:, k * SH:(k + 1) * SH]))
                for k in range(NCORES)]
    meta = dict(NPAD=NPAD, NBLK=NBLK, BPC=BPC, C=C, PC=PC, H=H, D=D,
                bp2f=float(np.asarray(bp2).reshape(-1)[0]))
    return meta, shared, per_core


def _build(meta):
    NPAD, NBLK, BPC, C, PC = (meta["NPAD"], meta["NBLK"], meta["BPC"],
                              meta["C"], meta["PC"])
    H, D, bp2f = meta["H"], meta["D"], meta["bp2f"]
    f32, f16, i32 = mybir.dt.float32, mybir.dt.float16, mybir.dt.int32
    AF = mybir.ActivationFunctionType
    Alu = mybir.AluOpType
    RG = [list(range(NCORES))]
    SH = BPC * PT

    nc = bacc.Bacc("TRN2", target_bir_lowering=False, debug=False,
                   num_devices=NCORES)

    # kernel I/O
    node_emb = nc.dram_tensor("node_emb", [SH, H], f16, kind="ExternalInput")
    type_emb = nc.dram_tensor("type_emb", [TE, H], f16, kind="ExternalInput")
    tyoh_d = nc.dram_tensor("types_oh", [TE, SH], f16, kind="ExternalInput")
    cols1_d = nc.dram_tensor("cols1", [PT, BPC * C], i32, kind="ExternalInput")
    cols2_d = nc.dram_tensor("cols2", [PT, BPC * C], i32, kind="ExternalInput")
    rid1_d = nc.dram_tensor("rid1", [PT, BPC * C], f32, kind="ExternalInput")
    val1_d = nc.dram_tensor("val1", [PT, BPC * C], f32, kind="ExternalInput")
    rid2_d = nc.dram_tensor("rid2", [PT, BPC * C], f32, kind="ExternalInput")
    val2_d = nc.dram_tensor("val2", [PT, BPC * C], f32, kind="ExternalInput")
    psrc_d = nc.dram_tensor("psrc", [PT, PC], i32, kind="ExternalInput")
    pdst_d = nc.dram_tensor("pdst", [PT, PC], i32, kind="ExternalInput")
    w1_d = nc.dram_tensor("w1", [H, H], f16, kind="ExternalInput")
    w2_d = nc.dram_tensor("w2", [H, D], f16, kind="ExternalInput")
    wp1a_d = nc.dram_tensor("wp1a", [D, D], f16, kind="ExternalInput")
    wp1b_d = nc.dram_tensor("wp1b", [D, D], f16, kind="ExternalInput")
    wp1c_d = nc.dram_tensor("wp1c", [D, D], f16, kind="ExternalInput")
    wp2_d = nc.dram_tensor("wp2", [D, 1], f16, kind="ExternalInput")
    b1bc_d = nc.dram_tensor("b1bc", [PT, H], f32, kind="ExternalInput")
    b2bc_d = nc.dram_tensor("b2bc", [PT, D], f32, kind="ExternalInput")
    bp1c_d = nc.dram_tensor("bp1col", [D, 1], f32, kind="ExternalInput")
    iota_d = nc.dram_tensor("iota", [PT, PT], f16, kind="ExternalInput")
    ident_d = nc.dram_tensor("ident", [PT, PT], f16, kind="ExternalInput")
    outp = nc.dram_tensor("out", [PC * PT, 1], f32, kind="ExternalOutput")

    # internal feature tables
    x_shard = nc.dram_tensor("x_shard", [SH, H], f16, kind="Internal")
    x_full = nc.dram_tensor("x_full", [NPAD, H], f16, kind="Internal",
                            addr_space="Shared")
    h_shard = nc.dram_tensor("h_shard", [SH, H], f16, kind="Internal")
    h_full = nc.dram_tensor("h_full", [NPAD, H], f16, kind="Internal",
                            addr_space="Shared")
    z_shard = nc.dram_tensor("z_shard", [SH, D], f16, kind="Internal")
    z_full = nc.dram_tensor("z_full", [NPAD, D], f16, kind="Internal",
                            addr_space="Shared")

    with tile.TileContext(nc) as tc:
        with (
            tc.tile_pool(name="const", bufs=1) as cpool,
            tc.tile_pool(name="idx", bufs=1) as ipool,
            tc.tile_pool(name="xne", bufs=3) as xnp,
            tc.tile_pool(name="gath", bufs=8) as gpool,
            tc.tile_pool(name="onep", bufs=8) as opool,
            tc.tile_pool(name="accs", bufs=3) as apool,
            tc.tile_pool(name="outs", bufs=3) as hpool,
            tc.tile_pool(name="pred", bufs=6) as ppool,
            tc.tile_pool(name="locp", bufs=BPC) as locpool,
            tc.tile_pool(name="ps_agg", bufs=3, space="PSUM") as ps_agg,
            tc.tile_pool(name="ps_out", bufs=3, space="PSUM") as ps_out,
        ):
            def sb(pool, dram, shape, dtype):
                t = pool.tile(shape, dtype, name=dram.name + "_sb")
                nc.sync.dma_start(t[:], dram[:])
                return t

            # resident SBUF state
            iota_sb = sb(cpool, iota_d, [PT, PT], f16)
            ident_sb = sb(cpool, ident_d, [PT, PT], f16)
            w1_sb = sb(cpool, w1_d, [H, H], f16)
            w2_sb = sb(cpool, w2_d, [H, D], f16)
            wp1a_sb = sb(cpool, wp1a_d, [D, D], f16)
            wp1b_sb = sb(cpool, wp1b_d, [D, D], f16)
            wp1c_sb = sb(cpool, wp1c_d, [D, D], f16)
            wp2_sb = sb(cpool, wp2_d, [D, 1], f16)
            b1bc_sb = sb(cpool, b1bc_d, [PT, H], f32)
            b2bc_sb = sb(cpool, b2bc_d, [PT, D], f32)
            bp1c_sb = sb(cpool, bp1c_d, [D, 1], f32)
            tyemb_sb = sb(cpool, type_emb, [TE, H], f16)
            tyoh_sb = sb(ipool, tyoh_d, [TE, SH], f16)
            cols1_sb = sb(ipool, cols1_d, [PT, BPC * C], i32)
            cols2_sb = sb(ipool, cols2_d, [PT, BPC * C], i32)
            rid1_sb = sb(ipool, rid1_d, [PT, BPC * C], f32)
            val1_sb = sb(ipool, val1_d, [PT, BPC * C], f32)
            rid2_sb = sb(ipool, rid2_d, [PT, BPC * C], f32)
            val2_sb = sb(ipool, val2_d, [PT, BPC * C], f32)
            psrc_sb = sb(ipool, psrc_d, [PT, PC], i32)
            pdst_sb = sb(ipool, pdst_d, [PT, PC], i32)

            # ---- Phase X (sharded): x = node_emb + types_oh.T @ type_emb ----
            SB = 7 if BPC % 7 == 0 else 1  # blocks per supertile
            assert BPC % SB == 0
            for st in range(BPC // SB):
                a = st * SB * PT
                ne = xnp.tile([PT, SB * H], f16, name="ne")
                nc.sync.dma_start(
                    ne[:].rearrange("p (b f) -> p b f", b=SB),
                    node_emb[a:a + SB * PT, :].rearrange(
                        "(b p) f -> p b f", p=PT))
                xs = xnp.tile([PT, SB * H], f16, name="xs")
                for b in range(SB):
                    blk = st * SB + b
                    t_ps = ps_out.tile([PT, H], f32, name="o_ps", tag="o_ps")
                    nc.tensor.matmul(
                        t_ps[:], lhsT=tyoh_sb[:, blk * PT:(blk + 1) * PT],
                        rhs=tyemb_sb[:], start=True, stop=True)
                    nc.vector.tensor_add(
                        xs[:, b * H:(b + 1) * H], ne[:, b * H:(b + 1) * H],
                        t_ps[:])
                nc.sync.dma_start(
                    x_shard[a:a + SB * PT, :].rearrange("(b p) f -> p b f", p=PT),
                    xs[:].rearrange("p (b f) -> p b f", b=SB))
            nc.gpsimd.collective_compute(
                "AllGather", Alu.bypass, replica_groups=RG,
                ins=[x_shard[:]], outs=[x_full[:]])

            # ---- GCN layer ----
            # Chunk 0 of every bucket holds core-local edges: gathered from
            # the (pre-AllGather) shard table into a dedicated per-bucket
            # buffer so these DMAs can fill the AllGather barrier window.
            def gcn_layer(shard_table, full_table, colsb, ridb, valb,
                          dst_shard, w_sb, bias_sb, Dout, relu, suffix):
                locs = []
                for blk in range(BPC):
                    lg = locpool.tile([PT, H], f16, name="loc" + suffix)
                    nc.gpsimd.indirect_dma_start(
                        out=lg[:], out_offset=None, in_=shard_table[:],
                        in_offset=bass.IndirectOffsetOnAxis(
                            ap=colsb[:, blk * C:blk * C + 1], axis=0))
                    locs.append(lg)
                for blk in range(BPC):
                    agg_ps = ps_agg.tile([PT, PT], f32, name="agg_ps")
                    for c in range(C):
                        g = blk * C + c
                        if c == 0:
                            xg = locs[blk]
                        else:
                            xg = gpool.tile([PT, H], f16, name="xg" + suffix)
                            nc.gpsimd.indirect_dma_start(
                                out=xg[:], out_offset=None, in_=full_table[:],
                                in_offset=bass.IndirectOffsetOnAxis(
                                    ap=colsb[:, g:g + 1], axis=0))
                        oh = opool.tile([PT, PT], f16, name="oh" + suffix)
                        nc.vector.tensor_scalar(
                            oh[:], iota_sb[:], ridb[:, g:g + 1],
                            valb[:, g:g + 1], op0=Alu.is_equal, op1=Alu.mult)
                        nc.tensor.matmul(agg_ps[:], lhsT=xg[:], rhs=oh[:],
                                         start=(c == 0), stop=(c == C - 1))
                    aggT_sb = apool.tile([PT, PT], f16, name="aggT" + suffix)
                    nc.vector.tensor_copy(aggT_sb[:], agg_ps[:])
                    o_ps = ps_out.tile([PT, Dout], f32, name="o_ps", tag="o_ps")
                    nc.tensor.matmul(o_ps[:], lhsT=aggT_sb[:], rhs=w_sb[:],
                                     start=True, stop=True)
                    o_sb = hpool.tile([PT, Dout], f16, name="osb" + suffix)
                    if relu:
                        ob = hpool.tile([PT, Dout], f32, name="ob" + suffix)
                        nc.vector.tensor_add(ob[:], o_ps[:], bias_sb[:])
                        nc.scalar.activation(o_sb[:], ob[:], AF.Relu)
                    else:
                        nc.vector.tensor_add(o_sb[:], o_ps[:], bias_sb[:])
                    nc.sync.dma_start(
                        dst_shard[blk * PT:(blk + 1) * PT, :], o_sb[:])

            gcn_layer(x_shard, x_full, cols1_sb, rid1_sb, val1_sb,
                      h_shard, w1_sb, b1bc_sb, H, relu=True, suffix="1")
            nc.gpsimd.collective_compute(
                "AllGather", Alu.bypass, replica_groups=RG,
                ins=[h_shard[:]], outs=[h_full[:]])

            gcn_layer(h_shard, h_full, cols2_sb, rid2_sb, val2_sb,
                      z_shard, w2_sb, b2bc_sb, D, relu=False, suffix="2")
            nc.gpsimd.collective_compute(
                "AllGather", Alu.bypass, replica_groups=RG,
                ins=[z_shard[:]], outs=[z_full[:]])

            # ---- predictor ----
            for pc in range(PC):
                sg = ppool.tile([PT, D], f16, name="sg")
                nc.gpsimd.indirect_dma_start(
                    out=sg[:], out_offset=None, in_=z_full[:],
                    in_offset=bass.IndirectOffsetOnAxis(
                        ap=psrc_sb[:, pc:pc + 1], axis=0))
                dg = ppool.tile([PT, D], f16, name="dg")
                nc.gpsimd.indirect_dma_start(
                    out=dg[:], out_offset=None, in_=z_full[:],
                    in_offset=bass.IndirectOffsetOnAxis(
                        ap=pdst_sb[:, pc:pc + 1], axis=0))
                sgt_ps = ps_out.tile([D, PT], f16, name="sgt_ps", tag="o_ps")
                nc.tensor.transpose(sgt_ps[:], sg[:], ident_sb[:])
                dgt_ps = ps_out.tile([D, PT], f16, name="dgt_ps", tag="o_ps")
                nc.tensor.transpose(dgt_ps[:], dg[:], ident_sb[:])
                sgt = ppool.tile([D, PT], f16, name="sgt")
                nc.vector.tensor_copy(sgt[:], sgt_ps[:])
                dgt = ppool.tile([D, PT], f16, name="dgt")
                nc.vector.tensor_copy(dgt[:], dgt_ps[:])
                sdt = ppool.tile([D, PT], f16, name="sdt")
                nc.vector.tensor_mul(sdt[:], sgt[:], dgt[:])
                yt_ps = ps_agg.tile([D, PT], f32, name="yt_ps", tag="agg_ps")
                nc.tensor.matmul(yt_ps[:], lhsT=wp1a_sb[:], rhs=sgt[:],
                                 start=True, stop=False)
                nc.tensor.matmul(yt_ps[:], lhsT=wp1b_sb[:], rhs=dgt[:],
                                 start=False, stop=False)
                nc.tensor.matmul(yt_ps[:], lhsT=wp1c_sb[:], rhs=sdt[:],
                                 start=False, stop=True)
                r_sb = ppool.tile([D, PT], f16, name="r_sb")
                nc.scalar.activation(r_sb[:], yt_ps[:], AF.Relu,
                                     bias=bp1c_sb[:])
                o2_ps = ps_out.tile([PT, 1], f32, name="o2_ps", tag="o_ps")
                nc.tensor.matmul(o2_ps[:], lhsT=r_sb[:], rhs=wp2_sb[:],
                                 start=True, stop=True)
                o2_sb = ppool.tile([PT, 1], f32, name="o2_sb")
                nc.scalar.activation(o2_sb[:], o2_ps[:], AF.Copy, bias=bp2f)
                nc.sync.dma_start(outp[pc * PT:(pc + 1) * PT, :], o2_sb[:])

    nc.compile()
    return nc


def kernel(**inputs) -> np.ndarray:
    meta, shared, per_core = _preprocess(**inputs)
    key = tuple(sorted(meta.items()))
    if key not in _prog_cache:
        _prog_cache[key] = _build(meta)
    nc = _prog_cache[key]
    in_maps = [dict(shared, **per_core[k]) for k in range(NCORES)]
    res = bass_utils.run_bass_kernel_spmd(
        nc, in_maps, core_ids=list(range(NCORES)))
    out = np.concatenate(
        [np.asarray(res.results[k]["out"])[:, 0] for k in range(NCORES)])
    return out.astype(np.float32)


# revision 20
# speedup vs baseline: 1.0450x; 1.0055x over previous
"""Trainium2 Bass kernel for a 2-layer GCN + link predictor (PrimeKG drug
repurposing GNN).

Strategy (8 NeuronCores, SPMD single program):
  - Nodes are permuted into 128-node "buckets" balanced by in-degree; each
    core owns NBLK/8 consecutive buckets (rows of the aggregation).
  - Edges are grouped by destination bucket and padded to chunks of 128.
    segment_sum is computed per bucket as a sequence of PE matmuls:
       aggT[f, n] += sum_e xgath[e, f] * onehot[e, n]
    where xgath is an indirect-DMA gather of source-node features and
    onehot[e, n] = vals[e] * (n == local_row[e]) built on the vector engine.
  - x = node_emb + type_onehot.T @ type_emb is built sharded (original node
    order, host-precomputed transposed one-hot) and AllGathered; h and z
    live in permuted order, sharded and AllGathered likewise.
  - Pairs are sharded by batch; the predictor gathers z rows, transposes on
    the PE, and runs the tiny MLP per 128-pair chunk.

Feature tables and matmuls are fp16 with f32 PSUM accumulation.
"""

import numpy as np

import concourse.bass as bass
import concourse.bacc as bacc
import concourse.tile as tile
import concourse.mybir as mybir
from concourse import bass_utils

PT = 128  # partitions
NCORES = 8
TE = 16   # padded type-embedding rows

F16 = np.float16

_prog_cache: dict = {}


def _preprocess(node_type_ids, adj_rows, adj_cols, adj_vals, pairs,
                node_emb, type_emb, W1, b1, W2, b2, Wp1, bp1, Wp2, bp2):
    N, H = node_emb.shape
    T = type_emb.shape[0]
    E = adj_rows.shape[0]
    P2 = pairs.shape[1]
    D = W2.shape[1]
    assert H == PT and T <= TE and Wp1.shape == (3 * D, D)

    NPB = PT * NCORES
    NPAD = -(-N // NPB) * NPB
    NBLK = NPAD // PT
    BPC = NBLK // NCORES

    rows = np.asarray(adj_rows).astype(np.int64)
    cols = np.asarray(adj_cols).astype(np.int64)
    vals = np.asarray(adj_vals).astype(np.float32)
    types = np.asarray(node_type_ids).astype(np.int64)

    # Degree-balanced bucket assignment: deal nodes (sorted by in-degree
    # desc) round-robin across the NBLK buckets, then repair-swap nodes
    # between heavy and light buckets to pull the max bucket load down to
    # the next multiple-of-128 boundary.
    deg = np.bincount(rows, minlength=N).astype(np.int64)
    deg_pad = np.zeros(NPAD, np.int64)
    deg_pad[:N] = deg
    order = np.argsort(-deg_pad, kind="stable")
    i = np.arange(NPAD)
    bucket_of_rank = i % NBLK
    slot_of_rank = i // NBLK
    bucket_of = np.empty(NPAD, np.int64)
    bucket_of[order] = bucket_of_rank
    loads = np.bincount(bucket_of, weights=deg_pad, minlength=NBLK).astype(
        np.int64)
    target = max(PT, int(-(-int(loads.max()) // PT) - 1) * PT)
    members = [list(order[b::NBLK][::-1]) for b in range(NBLK)]  # asc degree
    for _ in range(2000):
        hb = int(np.argmax(loads))
        if loads[hb] <= target:
            break
        lb = int(np.argmin(loads))
        done = False
        for mi in range(len(members[hb]) - 1, -1, -1):
            m = members[hb][mi]
            for li, l in enumerate(members[lb]):
                delta = deg_pad[m] - deg_pad[l]
                if delta <= 0:
                    break
                if loads[lb] + delta <= target:
                    members[hb][mi], members[lb][li] = l, m
                    loads[hb] -= delta
                    loads[lb] += delta
                    done = True
                    break
            if done:
                break
        if not done:
            break
    perm = np.empty(N, np.int64)
    for b in range(NBLK):
        for s, m in enumerate(members[b]):
            if m < N:
                perm[m] = b * PT + s

    prow = perm[rows]
    bkt = prow // PT
    rid = (prow % PT).astype(np.float32)
    cnt = np.bincount(bkt, minlength=NBLK)
    SH = BPC * PT

    # Per-layer edge layout, local-first: chunk 0 of each bucket holds only
    # edges whose gather source row lives in the processing core's shard
    # (gathered from the shard table before the AllGather completes);
    # remaining edges go to chunks 1.. with full-table ids.
    def layer_arrays(tid):
        owner = bkt // BPC
        local = (tid // SH) == owner
        eo = np.lexsort((~local, bkt))
        b2 = bkt[eo]
        l2 = local[eo]
        t2 = tid[eo]
        r2 = rid[eo]
        v2 = vals[eo]
        cnts = np.bincount(b2, minlength=NBLK)
        st = np.concatenate([[0], np.cumsum(cnts)[:-1]])
        pos = np.arange(E) - st[b2]
        nl = np.bincount(b2, weights=l2.astype(np.float64),
                         minlength=NBLK).astype(np.int64)
        nl0 = np.minimum(nl, PT)
        slot = np.where(l2 & (pos < PT), pos, PT + pos - nl0[b2])
        owner2 = b2 // BPC
        ids = np.where(slot < PT, t2 - owner2 * SH, t2).astype(np.int32)
        Cl = int(-(-int((PT + cnts - nl0).max()) // PT))
        return b2, slot, ids, r2, v2, Cl

    b2a, slot_a, ids_a, rid_a, val_a, C1 = layer_arrays(cols)
    b2b, slot_b, ids_b, rid_b, val_b, C2 = layer_arrays(perm[cols])
    C = max(C1, C2)
    CAP = C * PT

    def fill(b2, slot, ids, r2, v2):
        ec = np.zeros((NBLK, CAP), np.int32)
        er = np.zeros((NBLK, CAP), np.float32)
        ev = np.zeros((NBLK, CAP), np.float32)
        ec[b2, slot] = ids
        er[b2, slot] = r2
        ev[b2, slot] = v2
        return ec, er, ev

    ecol1, erid1, eval1 = fill(b2a, slot_a, ids_a, rid_a, val_a)
    ecol2, erid2, eval2 = fill(b2b, slot_b, ids_b, rid_b, val_b)

    def per_core_T(a):
        # [NBLK, C*PT] -> per-core [PT, BPC*C]; column blk*C+c, partition p
        # holds bucket (core*BPC+blk) edge slot c*128+p.
        out = []
        for k in range(NCORES):
            sub = a[k * BPC:(k + 1) * BPC].reshape(BPC, C, PT)
            out.append(np.ascontiguousarray(
                sub.transpose(2, 0, 1).reshape(PT, BPC * C)))
        return out

    cols1_k = per_core_T(ecol1)
    rid1_k = per_core_T(erid1)
    val1_k = per_core_T(eval1)
    cols2_k = per_core_T(ecol2)
    rid2_k = per_core_T(erid2)
    val2_k = per_core_T(eval2)

    PPC = P2 // NCORES
    assert PPC % PT == 0
    PC = PPC // PT
    pp = perm[np.asarray(pairs).astype(np.int64)]
    psrc_k = [np.ascontiguousarray(
        pp[0, k * PPC:(k + 1) * PPC].reshape(PC, PT).T.astype(np.int32))
        for k in range(NCORES)]
    pdst_k = [np.ascontiguousarray(
        pp[1, k * PPC:(k + 1) * PPC].reshape(PC, PT).T.astype(np.int32))
        for k in range(NCORES)]

    types_pad = np.zeros(NPAD, np.int64)
    types_pad[:N] = types
    types_oh_t = np.zeros((TE, NPAD), F16)  # transposed one-hot, exact 0/1
    types_oh_t[types_pad, np.arange(NPAD)] = 1.0

    node_emb_pad = np.zeros((NPAD, H), F16)
    node_emb_pad[:N] = np.asarray(node_emb, np.float32).astype(F16)
    type_emb_pad = np.zeros((TE, H), F16)
    type_emb_pad[:T] = np.asarray(type_emb, np.float32).astype(F16)

    SH = BPC * PT  # x-shard rows per core
    Wp1 = np.asarray(Wp1, np.float32)
    shared = dict(
        type_emb=type_emb_pad,
        w1=np.asarray(W1, np.float32).astype(F16),
        w2=np.asarray(W2, np.float32).astype(F16),
        wp1a=np.ascontiguousarray(Wp1[0:D]).astype(F16),
        wp1b=np.ascontiguousarray(Wp1[D:2 * D]).astype(F16),
        wp1c=np.ascontiguousarray(Wp1[2 * D:3 * D]).astype(F16),
        wp2=np.asarray(Wp2, np.float32).astype(F16),
        b1bc=np.ascontiguousarray(np.broadcast_to(
            np.asarray(b1, np.float32), (PT, H))),
        b2bc=np.ascontiguousarray(np.broadcast_to(
            np.asarray(b2, np.float32), (PT, D))),
        bp1col=np.asarray(bp1, np.float32).reshape(D, 1),
        iota=np.ascontiguousarray(np.broadcast_to(
            np.arange(PT, dtype=np.float32), (PT, PT))).astype(F16),
        ident=np.eye(PT, dtype=F16),
    )
    per_core = [dict(cols1=cols1_k[k], cols2=cols2_k[k], rid1=rid1_k[k],
                     val1=val1_k[k], rid2=rid2_k[k], val2=val2_k[k],
                     psrc=psrc_k[k], pdst=pdst_k[k],
                     node_emb=np.ascontiguousarray(
                         node_emb_pad[k * SH:(k + 1) * SH]),
                     types_oh=np.ascontiguousarray(
                         types_oh_t[:, k * SH:(k + 1) * SH]))
                for k in range(NCORES)]
    meta = dict(NPAD=NPAD, NBLK=NBLK, BPC=BPC, C=C, PC=PC, H=H, D=D,
                bp2f=float(np.asarray(bp2).reshape(-1)[0]))
    return meta, shared, per_core


def _build(meta):
    NPAD, NBLK, BPC, C, PC = (meta["NPAD"], meta["NBLK"], meta["BPC"],
                              meta["C"], meta["PC"])
    H, D, bp2f = meta["H"], meta["D"], meta["bp2f"]
    f32, f16, i32 = mybir.dt.float32, mybir.dt.float16, mybir.dt.int32
    AF = mybir.ActivationFunctionType
    Alu = mybir.AluOpType
    RG = [list(range(NCORES))]
    SH = BPC * PT

    nc = bacc.Bacc("TRN2", target_bir_lowering=False, debug=False,
                   num_devices=NCORES)

    # kernel I/O
    node_emb = nc.dram_tensor("node_emb", [SH, H], f16, kind="ExternalInput")
    type_emb = nc.dram_tensor("type_emb", [TE, H], f16, kind="ExternalInput")
    tyoh_d = nc.dram_tensor("types_oh", [TE, SH], f16, kind="ExternalInput")
    cols1_d = nc.dram_tensor("cols1", [PT, BPC * C], i32, kind="ExternalInput")
    cols2_d = nc.dram_tensor("cols2", [PT, BPC * C], i32, kind="ExternalInput")
    rid1_d = nc.dram_tensor("rid1", [PT, BPC * C], f32, kind="ExternalInput")
    val1_d = nc.dram_tensor("val1", [PT, BPC * C], f32, kind="ExternalInput")
    rid2_d = nc.dram_tensor("rid2", [PT, BPC * C], f32, kind="ExternalInput")
    val2_d = nc.dram_tensor("val2", [PT, BPC * C], f32, kind="ExternalInput")
    psrc_d = nc.dram_tensor("psrc", [PT, PC], i32, kind="ExternalInput")
    pdst_d = nc.dram_tensor("pdst", [PT, PC], i32, kind="ExternalInput")
    w1_d = nc.dram_tensor("w1", [H, H], f16, kind="ExternalInput")
    w2_d = nc.dram_tensor("w2", [H, D], f16, kind="ExternalInput")
    wp1a_d = nc.dram_tensor("wp1a", [D, D], f16, kind="ExternalInput")
    wp1b_d = nc.dram_tensor("wp1b", [D, D], f16, kind="ExternalInput")
    wp1c_d = nc.dram_tensor("wp1c", [D, D], f16, kind="ExternalInput")
    wp2_d = nc.dram_tensor("wp2", [D, 1], f16, kind="ExternalInput")
    b1bc_d = nc.dram_tensor("b1bc", [PT, H], f32, kind="ExternalInput")
    b2bc_d = nc.dram_tensor("b2bc", [PT, D], f32, kind="ExternalInput")
    bp1c_d = nc.dram_tensor("bp1col", [D, 1], f32, kind="ExternalInput")
    iota_d = nc.dram_tensor("iota", [PT, PT], f16, kind="ExternalInput")
    ident_d = nc.dram_tensor("ident", [PT, PT], f16, kind="ExternalInput")
    outp = nc.dram_tensor("out", [PC * PT, 1], f32, kind="ExternalOutput")

    # internal feature tables
    x_shard = nc.dram_tensor("x_shard", [SH, H], f16, kind="Internal")
    x_full = nc.dram_tensor("x_full", [NPAD, H], f16, kind="Internal",
                            addr_space="Shared")
    h_shard = nc.dram_tensor("h_shard", [SH, H], f16, kind="Internal")
    h_full = nc.dram_tensor("h_full", [NPAD, H], f16, kind="Internal",
                            addr_space="Shared")
    z_shard = nc.dram_tensor("z_shard", [SH, D], f16, kind="Internal")
    z_full = nc.dram_tensor("z_full", [NPAD, D], f16, kind="Internal",
                            addr_space="Shared")

    with tile.TileContext(nc) as tc:
        with (
            tc.tile_pool(name="const", bufs=1) as cpool,
            tc.tile_pool(name="idx", bufs=1) as ipool,
            tc.tile_pool(name="xne", bufs=3) as xnp,
            tc.tile_pool(name="gath", bufs=12) as gpool,
            tc.tile_pool(name="onep", bufs=8) as opool,
            tc.tile_pool(name="accs", bufs=3) as apool,
            tc.tile_pool(name="outs", bufs=3) as hpool,
            tc.tile_pool(name="pred", bufs=6) as ppool,
            tc.tile_pool(name="locp", bufs=BPC) as locpool,
            tc.tile_pool(name="ps_agg", bufs=3, space="PSUM") as ps_agg,
            tc.tile_pool(name="ps_out", bufs=3, space="PSUM") as ps_out,
            tc.tile_pool(name="ps_tr", bufs=2, space="PSUM") as ps_tr,
        ):
            def sb(pool, dram, shape, dtype):
                t = pool.tile(shape, dtype, name=dram.name + "_sb")
                nc.sync.dma_start(t[:], dram[:])
                return t

            # resident SBUF state
            iota_sb = sb(cpool, iota_d, [PT, PT], f16)
            ident_sb = sb(cpool, ident_d, [PT, PT], f16)
            w1_sb = sb(cpool, w1_d, [H, H], f16)
            w2_sb = sb(cpool, w2_d, [H, D], f16)
            wp1a_sb = sb(cpool, wp1a_d, [D, D], f16)
            wp1b_sb = sb(cpool, wp1b_d, [D, D], f16)
            wp1c_sb = sb(cpool, wp1c_d, [D, D], f16)
            wp2_sb = sb(cpool, wp2_d, [D, 1], f16)
            b1bc_sb = sb(cpool, b1bc_d, [PT, H], f32)
            b2bc_sb = sb(cpool, b2bc_d, [PT, D], f32)
            bp1c_sb = sb(cpool, bp1c_d, [D, 1], f32)
            tyemb_sb = sb(cpool, type_emb, [TE, H], f16)
            tyoh_sb = sb(ipool, tyoh_d, [TE, SH], f16)
            cols1_sb = sb(ipool, cols1_d, [PT, BPC * C], i32)
            cols2_sb = sb(ipool, cols2_d, [PT, BPC * C], i32)
            rid1_sb = sb(ipool, rid1_d, [PT, BPC * C], f32)
            val1_sb = sb(ipool, val1_d, [PT, BPC * C], f32)
            rid2_sb = sb(ipool, rid2_d, [PT, BPC * C], f32)
            val2_sb = sb(ipool, val2_d, [PT, BPC * C], f32)
            psrc_sb = sb(ipool, psrc_d, [PT, PC], i32)
            pdst_sb = sb(ipool, pdst_d, [PT, PC], i32)

            # ---- Phase X (sharded): x = node_emb + types_oh.T @ type_emb ----
            SB = 7 if BPC % 7 == 0 else 1  # blocks per supertile
            assert BPC % SB == 0
            for st in range(BPC // SB):
                a = st * SB * PT
                ne = xnp.tile([PT, SB * H], f16, name="ne")
                nc.sync.dma_start(
                    ne[:].rearrange("p (b f) -> p b f", b=SB),
                    node_emb[a:a + SB * PT, :].rearrange(
                        "(b p) f -> p b f", p=PT))
                xs = xnp.tile([PT, SB * H], f16, name="xs")
                for b in range(SB):
                    blk = st * SB + b
                    t_ps = ps_out.tile([PT, H], f32, name="o_ps", tag="o_ps")
                    nc.tensor.matmul(
                        t_ps[:], lhsT=tyoh_sb[:, blk * PT:(blk + 1) * PT],
                        rhs=tyemb_sb[:], start=True, stop=True)
                    nc.vector.tensor_add(
                        xs[:, b * H:(b + 1) * H], ne[:, b * H:(b + 1) * H],
                        t_ps[:])
                nc.sync.dma_start(
                    x_shard[a:a + SB * PT, :].rearrange("(b p) f -> p b f", p=PT),
                    xs[:].rearrange("p (b f) -> p b f", b=SB))
            nc.gpsimd.collective_compute(
                "AllGather", Alu.bypass, replica_groups=RG,
                ins=[x_shard[:]], outs=[x_full[:]])

            # ---- GCN layer ----
            # Chunk 0 of every bucket holds core-local edges: gathered from
            # the (pre-AllGather) shard table into a dedicated per-bucket
            # buffer so these DMAs can fill the AllGather barrier window.
            def gcn_layer(shard_table, full_table, colsb, ridb, valb,
                          dst_shard, w_sb, bias_sb, Dout, relu, suffix):
                locs = []
                for blk in range(BPC):
                    lg = locpool.tile([PT, H], f16, name="loc" + suffix)
                    nc.gpsimd.indirect_dma_start(
                        out=lg[:], out_offset=None, in_=shard_table[:],
                        in_offset=bass.IndirectOffsetOnAxis(
                            ap=colsb[:, blk * C:blk * C + 1], axis=0))
                    locs.append(lg)
                for blk in range(BPC):
                    agg_ps = ps_agg.tile([PT, PT], f32, name="agg_ps")
                    for c in range(C):
                        g = blk * C + c
                        if c == 0:
                            xg = locs[blk]
                        else:
                            xg = gpool.tile([PT, H], f16, name="xg" + suffix)
                            nc.gpsimd.indirect_dma_start(
                                out=xg[:], out_offset=None, in_=full_table[:],
                                in_offset=bass.IndirectOffsetOnAxis(
                                    ap=colsb[:, g:g + 1], axis=0))
                        oh = opool.tile([PT, PT], f16, name="oh" + suffix)
                        nc.vector.tensor_scalar(
                            oh[:], iota_sb[:], ridb[:, g:g + 1],
                            valb[:, g:g + 1], op0=Alu.is_equal, op1=Alu.mult)
                        nc.tensor.matmul(agg_ps[:], lhsT=xg[:], rhs=oh[:],
                                         start=(c == 0), stop=(c == C - 1))
                    aggT_sb = apool.tile([PT, PT], f16, name="aggT" + suffix)
                    nc.vector.tensor_copy(aggT_sb[:], agg_ps[:])
                    o_ps = ps_out.tile([PT, Dout], f32, name="o_ps", tag="o_ps")
                    nc.tensor.matmul(o_ps[:], lhsT=aggT_sb[:], rhs=w_sb[:],
                                     start=True, stop=True)
                    o_sb = hpool.tile([PT, Dout], f16, name="osb" + suffix)
                    if relu:
                        ob = hpool.tile([PT, Dout], f32, name="ob" + suffix)
                        nc.vector.tensor_add(ob[:], o_ps[:], bias_sb[:])
                        nc.scalar.activation(o_sb[:], ob[:], AF.Relu)
                    else:
                        nc.vector.tensor_add(o_sb[:], o_ps[:], bias_sb[:])
                    nc.sync.dma_start(
                        dst_shard[blk * PT:(blk + 1) * PT, :], o_sb[:])

            gcn_layer(x_shard, x_full, cols1_sb, rid1_sb, val1_sb,
                      h_shard, w1_sb, b1bc_sb, H, relu=True, suffix="1")
            nc.gpsimd.collective_compute(
                "AllGather", Alu.bypass, replica_groups=RG,
                ins=[h_shard[:]], outs=[h_full[:]])

            gcn_layer(h_shard, h_full, cols2_sb, rid2_sb, val2_sb,
                      z_shard, w2_sb, b2bc_sb, D, relu=False, suffix="2")
            nc.gpsimd.collective_compute(
                "AllGather", Alu.bypass, replica_groups=RG,
                ins=[z_shard[:]], outs=[z_full[:]])

            # ---- predictor ----
            for pc in range(PC):
                sg = ppool.tile([PT, D], f16, name="sg")
                nc.gpsimd.indirect_dma_start(
                    out=sg[:], out_offset=None, in_=z_full[:],
                    in_offset=bass.IndirectOffsetOnAxis(
                        ap=psrc_sb[:, pc:pc + 1], axis=0))
                dg = ppool.tile([PT, D], f16, name="dg")
                nc.gpsimd.indirect_dma_start(
                    out=dg[:], out_offset=None, in_=z_full[:],
                    in_offset=bass.IndirectOffsetOnAxis(
                        ap=pdst_sb[:, pc:pc + 1], axis=0))
                sgt_ps = ps_tr.tile([D, PT], f16, name="sgt_ps", tag="tps")
                nc.tensor.transpose(sgt_ps[:], sg[:], ident_sb[:])
                dgt_ps = ps_tr.tile([D, PT], f16, name="dgt_ps", tag="tps")
                nc.tensor.transpose(dgt_ps[:], dg[:], ident_sb[:])
                sgt = ppool.tile([D, PT], f16, name="sgt")
                nc.vector.tensor_copy(sgt[:], sgt_ps[:])
                dgt = ppool.tile([D, PT], f16, name="dgt")
                nc.vector.tensor_copy(dgt[:], dgt_ps[:])
                sdt = ppool.tile([D, PT], f16, name="sdt")
                nc.vector.tensor_mul(sdt[:], sgt[:], dgt[:])
                yt_ps = ps_agg.tile([D, PT], f32, name="yt_ps", tag="agg_ps")
                nc.tensor.matmul(yt_ps[:], lhsT=wp1a_sb[:], rhs=sgt[:],
                                 start=True, stop=False)
                nc.tensor.matmul(yt_ps[:], lhsT=wp1b_sb[:], rhs=dgt[:],
                                 start=False, stop=False)
                nc.tensor.matmul(yt_ps[:], lhsT=wp1c_sb[:], rhs=sdt[:],
                                 start=False, stop=True)
                r_sb = ppool.tile([D, PT], f16, name="r_sb")
                nc.scalar.activation(r_sb[:], yt_ps[:], AF.Relu,
                                     bias=bp1c_sb[:])
                o2_ps = ps_out.tile([PT, 1], f32, name="o2_ps", tag="o_ps")
                nc.tensor.matmul(o2_ps[:], lhsT=r_sb[:], rhs=wp2_sb[:],
                                 start=True, stop=True)
                o2_sb = ppool.tile([PT, 1], f32, name="o2_sb")
                nc.scalar.activation(o2_sb[:], o2_ps[:], AF.Copy, bias=bp2f)
                nc.sync.dma_start(outp[pc * PT:(pc + 1) * PT, :], o2_sb[:])

    nc.compile()
    return nc


def kernel(**inputs) -> np.ndarray:
    meta, shared, per_core = _preprocess(**inputs)
    key = tuple(sorted(meta.items()))
    if key not in _prog_cache:
        _prog_cache[key] = _build(meta)
    nc = _prog_cache[key]
    in_maps = [dict(shared, **per_core[k]) for k in range(NCORES)]
    res = bass_utils.run_bass_kernel_spmd(
        nc, in_maps, core_ids=list(range(NCORES)))
    out = np.concatenate(
        [np.asarray(res.results[k]["out"])[:, 0] for k in range(NCORES)])
    return out.astype(np.float32)


# revision 21
# speedup vs baseline: 1.0517x; 1.0064x over previous
"""Trainium2 Bass kernel for a 2-layer GCN + link predictor (PrimeKG drug
repurposing GNN).

Strategy (8 NeuronCores, SPMD single program):
  - Nodes are permuted into 128-node "buckets" balanced by in-degree; each
    core owns NBLK/8 consecutive buckets (rows of the aggregation).
  - Edges are grouped by destination bucket and padded to chunks of 128.
    segment_sum is computed per bucket as a sequence of PE matmuls:
       aggT[f, n] += sum_e xgath[e, f] * onehot[e, n]
    where xgath is an indirect-DMA gather of source-node features and
    onehot[e, n] = vals[e] * (n == local_row[e]) built on the vector engine.
  - x = node_emb + type_onehot.T @ type_emb is built sharded (original node
    order, host-precomputed transposed one-hot) and AllGathered; h and z
    live in permuted order, sharded and AllGathered likewise.
  - Pairs are sharded by batch; the predictor gathers z rows, transposes on
    the PE, and runs the tiny MLP per 128-pair chunk.

Feature tables and matmuls are fp16 with f32 PSUM accumulation.
"""

import numpy as np

import concourse.bass as bass
import concourse.bacc as bacc
import concourse.tile as tile
import concourse.mybir as mybir
from concourse import bass_utils

PT = 128  # partitions
NCORES = 8
TE = 16   # padded type-embedding rows

F16 = np.float16

_prog_cache: dict = {}


def _preprocess(node_type_ids, adj_rows, adj_cols, adj_vals, pairs,
                node_emb, type_emb, W1, b1, W2, b2, Wp1, bp1, Wp2, bp2):
    N, H = node_emb.shape
    T = type_emb.shape[0]
    E = adj_rows.shape[0]
    P2 = pairs.shape[1]
    D = W2.shape[1]
    assert H == PT and T <= TE and Wp1.shape == (3 * D, D)

    NPB = PT * NCORES
    NPAD = -(-N // NPB) * NPB
    NBLK = NPAD // PT
    BPC = NBLK // NCORES

    rows = np.asarray(adj_rows).astype(np.int64)
    cols = np.asarray(adj_cols).astype(np.int64)
    vals = np.asarray(adj_vals).astype(np.float32)
    types = np.asarray(node_type_ids).astype(np.int64)

    # Degree-balanced bucket assignment: deal nodes (sorted by in-degree
    # desc) round-robin across the NBLK buckets, then repair-swap nodes
    # between heavy and light buckets to pull the max bucket load down to
    # the next multiple-of-128 boundary.
    deg = np.bincount(rows, minlength=N).astype(np.int64)
    deg_pad = np.zeros(NPAD, np.int64)
    deg_pad[:N] = deg
    order = np.argsort(-deg_pad, kind="stable")
    i = np.arange(NPAD)
    bucket_of_rank = i % NBLK
    slot_of_rank = i // NBLK
    bucket_of = np.empty(NPAD, np.int64)
    bucket_of[order] = bucket_of_rank
    loads = np.bincount(bucket_of, weights=deg_pad, minlength=NBLK).astype(
        np.int64)
    target = max(PT, int(-(-int(loads.max()) // PT) - 1) * PT)
    members = [list(order[b::NBLK][::-1]) for b in range(NBLK)]  # asc degree
    for _ in range(2000):
        hb = int(np.argmax(loads))
        if loads[hb] <= target:
            break
        lb = int(np.argmin(loads))
        done = False
        for mi in range(len(members[hb]) - 1, -1, -1):
            m = members[hb][mi]
            for li, l in enumerate(members[lb]):
                delta = deg_pad[m] - deg_pad[l]
                if delta <= 0:
                    break
                if loads[lb] + delta <= target:
                    members[hb][mi], members[lb][li] = l, m
                    loads[hb] -= delta
                    loads[lb] += delta
                    done = True
                    break
            if done:
                break
        if not done:
            break
    perm = np.empty(N, np.int64)
    for b in range(NBLK):
        for s, m in enumerate(members[b]):
            if m < N:
                perm[m] = b * PT + s

    prow = perm[rows]
    bkt = prow // PT
    rid = (prow % PT).astype(np.float32)
    cnt = np.bincount(bkt, minlength=NBLK)
    SH = BPC * PT

    # Per-layer edge layout, local-first: chunk 0 of each bucket holds only
    # edges whose gather source row lives in the processing core's shard
    # (gathered from the shard table before the AllGather completes);
    # remaining edges go to chunks 1.. with full-table ids.
    def layer_arrays(tid):
        owner = bkt // BPC
        local = (tid // SH) == owner
        eo = np.lexsort((~local, bkt))
        b2 = bkt[eo]
        l2 = local[eo]
        t2 = tid[eo]
        r2 = rid[eo]
        v2 = vals[eo]
        cnts = np.bincount(b2, minlength=NBLK)
        st = np.concatenate([[0], np.cumsum(cnts)[:-1]])
        pos = np.arange(E) - st[b2]
        nl = np.bincount(b2, weights=l2.astype(np.float64),
                         minlength=NBLK).astype(np.int64)
        nl0 = np.minimum(nl, PT)
        slot = np.where(l2 & (pos < PT), pos, PT + pos - nl0[b2])
        owner2 = b2 // BPC
        ids = np.where(slot < PT, t2 - owner2 * SH, t2).astype(np.int32)
        Cl = int(-(-int((PT + cnts - nl0).max()) // PT))
        return b2, slot, ids, r2, v2, Cl

    b2a, slot_a, ids_a, rid_a, val_a, C1 = layer_arrays(cols)
    b2b, slot_b, ids_b, rid_b, val_b, C2 = layer_arrays(perm[cols])
    C = max(C1, C2)
    CAP = C * PT

    def fill(b2, slot, ids, r2, v2):
        ec = np.zeros((NBLK, CAP), np.int32)
        er = np.zeros((NBLK, CAP), np.float32)
        ev = np.zeros((NBLK, CAP), np.float32)
        ec[b2, slot] = ids
        er[b2, slot] = r2
        ev[b2, slot] = v2
        return ec, er, ev

    ecol1, erid1, eval1 = fill(b2a, slot_a, ids_a, rid_a, val_a)
    ecol2, erid2, eval2 = fill(b2b, slot_b, ids_b, rid_b, val_b)

    def per_core_T(a):
        # [NBLK, C*PT] -> per-core [PT, BPC*C]; column blk*C+c, partition p
        # holds bucket (core*BPC+blk) edge slot c*128+p.
        out = []
        for k in range(NCORES):
            sub = a[k * BPC:(k + 1) * BPC].reshape(BPC, C, PT)
            out.append(np.ascontiguousarray(
                sub.transpose(2, 0, 1).reshape(PT, BPC * C)))
        return out

    cols1_k = per_core_T(ecol1)
    rid1_k = per_core_T(erid1)
    val1_k = per_core_T(eval1)
    cols2_k = per_core_T(ecol2)
    rid2_k = per_core_T(erid2)
    val2_k = per_core_T(eval2)

    PPC = P2 // NCORES
    assert PPC % PT == 0
    PC = PPC // PT
    pp = perm[np.asarray(pairs).astype(np.int64)]
    psrc_k = [np.ascontiguousarray(
        pp[0, k * PPC:(k + 1) * PPC].reshape(PC, PT).T.astype(np.int32))
        for k in range(NCORES)]
    pdst_k = [np.ascontiguousarray(
        pp[1, k * PPC:(k + 1) * PPC].reshape(PC, PT).T.astype(np.int32))
        for k in range(NCORES)]

    types_pad = np.zeros(NPAD, np.int64)
    types_pad[:N] = types
    types_oh_t = np.zeros((TE, NPAD), F16)  # transposed one-hot, exact 0/1
    types_oh_t[types_pad, np.arange(NPAD)] = 1.0

    node_emb_pad = np.zeros((NPAD, H), F16)
    node_emb_pad[:N] = np.asarray(node_emb, np.float32).astype(F16)
    type_emb_pad = np.zeros((TE, H), F16)
    type_emb_pad[:T] = np.asarray(type_emb, np.float32).astype(F16)

    SH = BPC * PT  # x-shard rows per core
    Wp1 = np.asarray(Wp1, np.float32)
    shared = dict(
        type_emb=type_emb_pad,
        w1=np.asarray(W1, np.float32).astype(F16),
        w2=np.asarray(W2, np.float32).astype(F16),
        wp1a=np.ascontiguousarray(Wp1[0:D]).astype(F16),
        wp1b=np.ascontiguousarray(Wp1[D:2 * D]).astype(F16),
        wp1c=np.ascontiguousarray(Wp1[2 * D:3 * D]).astype(F16),
        wp2=np.asarray(Wp2, np.float32).astype(F16),
        b1bc=np.ascontiguousarray(np.broadcast_to(
            np.asarray(b1, np.float32), (PT, H))),
        b2bc=np.ascontiguousarray(np.broadcast_to(
            np.asarray(b2, np.float32), (PT, D))),
        bp1col=np.asarray(bp1, np.float32).reshape(D, 1),
        iota=np.ascontiguousarray(np.broadcast_to(
            np.arange(PT, dtype=np.float32), (PT, PT))).astype(F16),
        ident=np.eye(PT, dtype=F16),
    )
    per_core = [dict(cols1=cols1_k[k], cols2=cols2_k[k], rid1=rid1_k[k],
                     val1=val1_k[k], rid2=rid2_k[k], val2=val2_k[k],
                     psrc=psrc_k[k], pdst=pdst_k[k],
                     node_emb=np.ascontiguousarray(
                         node_emb_pad[k * SH:(k + 1) * SH]),
                     types_oh=np.ascontiguousarray(
                         types_oh_t[:, k * SH:(k + 1) * SH]))
                for k in range(NCORES)]
    meta = dict(NPAD=NPAD, NBLK=NBLK, BPC=BPC, C=C, PC=PC, H=H, D=D,
                bp2f=float(np.asarray(bp2).reshape(-1)[0]))
    return meta, shared, per_core


def _build(meta):
    NPAD, NBLK, BPC, C, PC = (meta["NPAD"], meta["NBLK"], meta["BPC"],
                              meta["C"], meta["PC"])
    H, D, bp2f = meta["H"], meta["D"], meta["bp2f"]
    f32, f16, i32 = mybir.dt.float32, mybir.dt.float16, mybir.dt.int32
    AF = mybir.ActivationFunctionType
    Alu = mybir.AluOpType
    RG = [list(range(NCORES))]
    SH = BPC * PT

    nc = bacc.Bacc("TRN2", target_bir_lowering=False, debug=False,
                   num_devices=NCORES, dynamic_dma_scratch_size=32768)

    # kernel I/O
    node_emb = nc.dram_tensor("node_emb", [SH, H], f16, kind="ExternalInput")
    type_emb = nc.dram_tensor("type_emb", [TE, H], f16, kind="ExternalInput")
    tyoh_d = nc.dram_tensor("types_oh", [TE, SH], f16, kind="ExternalInput")
    cols1_d = nc.dram_tensor("cols1", [PT, BPC * C], i32, kind="ExternalInput")
    cols2_d = nc.dram_tensor("cols2", [PT, BPC * C], i32, kind="ExternalInput")
    rid1_d = nc.dram_tensor("rid1", [PT, BPC * C], f32, kind="ExternalInput")
    val1_d = nc.dram_tensor("val1", [PT, BPC * C], f32, kind="ExternalInput")
    rid2_d = nc.dram_tensor("rid2", [PT, BPC * C], f32, kind="ExternalInput")
    val2_d = nc.dram_tensor("val2", [PT, BPC * C], f32, kind="ExternalInput")
    psrc_d = nc.dram_tensor("psrc", [PT, PC], i32, kind="ExternalInput")
    pdst_d = nc.dram_tensor("pdst", [PT, PC], i32, kind="ExternalInput")
    w1_d = nc.dram_tensor("w1", [H, H], f16, kind="ExternalInput")
    w2_d = nc.dram_tensor("w2", [H, D], f16, kind="ExternalInput")
    wp1a_d = nc.dram_tensor("wp1a", [D, D], f16, kind="ExternalInput")
    wp1b_d = nc.dram_tensor("wp1b", [D, D], f16, kind="ExternalInput")
    wp1c_d = nc.dram_tensor("wp1c", [D, D], f16, kind="ExternalInput")
    wp2_d = nc.dram_tensor("wp2", [D, 1], f16, kind="ExternalInput")
    b1bc_d = nc.dram_tensor("b1bc", [PT, H], f32, kind="ExternalInput")
    b2bc_d = nc.dram_tensor("b2bc", [PT, D], f32, kind="ExternalInput")
    bp1c_d = nc.dram_tensor("bp1col", [D, 1], f32, kind="ExternalInput")
    iota_d = nc.dram_tensor("iota", [PT, PT], f16, kind="ExternalInput")
    ident_d = nc.dram_tensor("ident", [PT, PT], f16, kind="ExternalInput")
    outp = nc.dram_tensor("out", [PC * PT, 1], f32, kind="ExternalOutput")

    # internal feature tables
    x_shard = nc.dram_tensor("x_shard", [SH, H], f16, kind="Internal")
    x_full = nc.dram_tensor("x_full", [NPAD, H], f16, kind="Internal",
                            addr_space="Shared")
    h_shard = nc.dram_tensor("h_shard", [SH, H], f16, kind="Internal")
    h_full = nc.dram_tensor("h_full", [NPAD, H], f16, kind="Internal",
                            addr_space="Shared")
    z_shard = nc.dram_tensor("z_shard", [SH, D], f16, kind="Internal")
    z_full = nc.dram_tensor("z_full", [NPAD, D], f16, kind="Internal",
                            addr_space="Shared")

    with tile.TileContext(nc) as tc:
        with (
            tc.tile_pool(name="const", bufs=1) as cpool,
            tc.tile_pool(name="idx", bufs=1) as ipool,
            tc.tile_pool(name="xne", bufs=3) as xnp,
            tc.tile_pool(name="gath", bufs=12) as gpool,
            tc.tile_pool(name="onep", bufs=8) as opool,
            tc.tile_pool(name="accs", bufs=3) as apool,
            tc.tile_pool(name="outs", bufs=3) as hpool,
            tc.tile_pool(name="pred", bufs=6) as ppool,
            tc.tile_pool(name="locp", bufs=BPC) as locpool,
            tc.tile_pool(name="ps_agg", bufs=3, space="PSUM") as ps_agg,
            tc.tile_pool(name="ps_out", bufs=3, space="PSUM") as ps_out,
            tc.tile_pool(name="ps_tr", bufs=2, space="PSUM") as ps_tr,
        ):
            def sb(pool, dram, shape, dtype):
                t = pool.tile(shape, dtype, name=dram.name + "_sb")
                nc.sync.dma_start(t[:], dram[:])
                return t

            # resident SBUF state
            iota_sb = sb(cpool, iota_d, [PT, PT], f16)
            ident_sb = sb(cpool, ident_d, [PT, PT], f16)
            w1_sb = sb(cpool, w1_d, [H, H], f16)
            w2_sb = sb(cpool, w2_d, [H, D], f16)
            wp1a_sb = sb(cpool, wp1a_d, [D, D], f16)
            wp1b_sb = sb(cpool, wp1b_d, [D, D], f16)
            wp1c_sb = sb(cpool, wp1c_d, [D, D], f16)
            wp2_sb = sb(cpool, wp2_d, [D, 1], f16)
            b1bc_sb = sb(cpool, b1bc_d, [PT, H], f32)
            b2bc_sb = sb(cpool, b2bc_d, [PT, D], f32)
            bp1c_sb = sb(cpool, bp1c_d, [D, 1], f32)
            tyemb_sb = sb(cpool, type_emb, [TE, H], f16)
            tyoh_sb = sb(ipool, tyoh_d, [TE, SH], f16)
            cols1_sb = sb(ipool, cols1_d, [PT, BPC * C], i32)
            cols2_sb = sb(ipool, cols2_d, [PT, BPC * C], i32)
            rid1_sb = sb(ipool, rid1_d, [PT, BPC * C], f32)
            val1_sb = sb(ipool, val1_d, [PT, BPC * C], f32)
            rid2_sb = sb(ipool, rid2_d, [PT, BPC * C], f32)
            val2_sb = sb(ipool, val2_d, [PT, BPC * C], f32)
            psrc_sb = sb(ipool, psrc_d, [PT, PC], i32)
            pdst_sb = sb(ipool, pdst_d, [PT, PC], i32)

            # ---- Phase X (sharded): x = node_emb + types_oh.T @ type_emb ----
            SB = 7 if BPC % 7 == 0 else 1  # blocks per supertile
            assert BPC % SB == 0
            for st in range(BPC // SB):
                a = st * SB * PT
                ne = xnp.tile([PT, SB * H], f16, name="ne")
                nc.sync.dma_start(
                    ne[:].rearrange("p (b f) -> p b f", b=SB),
                    node_emb[a:a + SB * PT, :].rearrange(
                        "(b p) f -> p b f", p=PT))
                xs = xnp.tile([PT, SB * H], f16, name="xs")
                for b in range(SB):
                    blk = st * SB + b
                    t_ps = ps_out.tile([PT, H], f32, name="o_ps", tag="o_ps")
                    nc.tensor.matmul(
                        t_ps[:], lhsT=tyoh_sb[:, blk * PT:(blk + 1) * PT],
                        rhs=tyemb_sb[:], start=True, stop=True)
                    nc.vector.tensor_add(
                        xs[:, b * H:(b + 1) * H], ne[:, b * H:(b + 1) * H],
                        t_ps[:])
                nc.sync.dma_start(
                    x_shard[a:a + SB * PT, :].rearrange("(b p) f -> p b f", p=PT),
                    xs[:].rearrange("p (b f) -> p b f", b=SB))
            nc.gpsimd.collective_compute(
                "AllGather", Alu.bypass, replica_groups=RG,
                ins=[x_shard[:]], outs=[x_full[:]])

            # ---- GCN layer ----
            # Chunk 0 of every bucket holds core-local edges: gathered from
            # the (pre-AllGather) shard table into a dedicated per-bucket
            # buffer so these DMAs can fill the AllGather barrier window.
            def gcn_layer(shard_table, full_table, colsb, ridb, valb,
                          dst_shard, w_sb, bias_sb, Dout, relu, suffix):
                locs = []
                for blk in range(BPC):
                    lg = locpool.tile([PT, H], f16, name="loc" + suffix)
                    nc.gpsimd.indirect_dma_start(
                        out=lg[:], out_offset=None, in_=shard_table[:],
                        in_offset=bass.IndirectOffsetOnAxis(
                            ap=colsb[:, blk * C:blk * C + 1], axis=0))
                    locs.append(lg)
                for blk in range(BPC):
                    agg_ps = ps_agg.tile([PT, PT], f32, name="agg_ps")
                    for c in range(C):
                        g = blk * C + c
                        if c == 0:
                            xg = locs[blk]
                        else:
                            xg = gpool.tile([PT, H], f16, name="xg" + suffix)
                            nc.gpsimd.indirect_dma_start(
                                out=xg[:], out_offset=None, in_=full_table[:],
                                in_offset=bass.IndirectOffsetOnAxis(
                                    ap=colsb[:, g:g + 1], axis=0))
                        oh = opool.tile([PT, PT], f16, name="oh" + suffix)
                        nc.vector.tensor_scalar(
                            oh[:], iota_sb[:], ridb[:, g:g + 1],
                            valb[:, g:g + 1], op0=Alu.is_equal, op1=Alu.mult)
                        nc.tensor.matmul(agg_ps[:], lhsT=xg[:], rhs=oh[:],
                                         start=(c == 0), stop=(c == C - 1))
                    aggT_sb = apool.tile([PT, PT], f16, name="aggT" + suffix)
                    nc.vector.tensor_copy(aggT_sb[:], agg_ps[:])
                    o_ps = ps_out.tile([PT, Dout], f32, name="o_ps", tag="o_ps")
                    nc.tensor.matmul(o_ps[:], lhsT=aggT_sb[:], rhs=w_sb[:],
                                     start=True, stop=True)
                    o_sb = hpool.tile([PT, Dout], f16, name="osb" + suffix)
                    if relu:
                        ob = hpool.tile([PT, Dout], f32, name="ob" + suffix)
                        nc.vector.tensor_add(ob[:], o_ps[:], bias_sb[:])
                        nc.scalar.activation(o_sb[:], ob[:], AF.Relu)
                    else:
                        nc.vector.tensor_add(o_sb[:], o_ps[:], bias_sb[:])
                    nc.sync.dma_start(
                        dst_shard[blk * PT:(blk + 1) * PT, :], o_sb[:])

            gcn_layer(x_shard, x_full, cols1_sb, rid1_sb, val1_sb,
                      h_shard, w1_sb, b1bc_sb, H, relu=True, suffix="1")
            nc.gpsimd.collective_compute(
                "AllGather", Alu.bypass, replica_groups=RG,
                ins=[h_shard[:]], outs=[h_full[:]])

            gcn_layer(h_shard, h_full, cols2_sb, rid2_sb, val2_sb,
                      z_shard, w2_sb, b2bc_sb, D, relu=False, suffix="2")
            nc.gpsimd.collective_compute(
                "AllGather", Alu.bypass, replica_groups=RG,
                ins=[z_shard[:]], outs=[z_full[:]])

            # ---- predictor ----
            for pc in range(PC):
                sg = ppool.tile([PT, D], f16, name="sg")
                nc.gpsimd.indirect_dma_start(
                    out=sg[:], out_offset=None, in_=z_full[:],
                    in_offset=bass.IndirectOffsetOnAxis(
                        ap=psrc_sb[:, pc:pc + 1], axis=0))
                dg = ppool.tile([PT, D], f16, name="dg")
                nc.gpsimd.indirect_dma_start(
                    out=dg[:], out_offset=None, in_=z_full[:],
                    in_offset=bass.IndirectOffsetOnAxis(
                        ap=pdst_sb[:, pc:pc + 1], axis=0))
                sgt_ps = ps_tr.tile([D, PT], f16, name="sgt_ps", tag="tps")
                nc.tensor.transpose(sgt_ps[:], sg[:], ident_sb[:])
                dgt_ps = ps_tr.tile([D, PT], f16, name="dgt_ps", tag="tps")
                nc.tensor.transpose(dgt_ps[:], dg[:], ident_sb[:])
                sgt = ppool.tile([D, PT], f16, name="sgt")
                nc.vector.tensor_copy(sgt[:], sgt_ps[:])
                dgt = ppool.tile([D, PT], f16, name="dgt")
                nc.vector.tensor_copy(dgt[:], dgt_ps[:])
                sdt = ppool.tile([D, PT], f16, name="sdt")
                nc.vector.tensor_mul(sdt[:], sgt[:], dgt[:])
                yt_ps = ps_agg.tile([D, PT], f32, name="yt_ps", tag="agg_ps")
                nc.tensor.matmul(yt_ps[:], lhsT=wp1a_sb[:], rhs=sgt[:],
                                 start=True, stop=False)
                nc.tensor.matmul(yt_ps[:], lhsT=wp1b_sb[:], rhs=dgt[:],
                                 start=False, stop=False)
                nc.tensor.matmul(yt_ps[:], lhsT=wp1c_sb[:], rhs=sdt[:],
                                 start=False, stop=True)
                r_sb = ppool.tile([D, PT], f16, name="r_sb")
                nc.scalar.activation(r_sb[:], yt_ps[:], AF.Relu,
                                     bias=bp1c_sb[:])
                o2_ps = ps_out.tile([PT, 1], f32, name="o2_ps", tag="o_ps")
                nc.tensor.matmul(o2_ps[:], lhsT=r_sb[:], rhs=wp2_sb[:],
                                 start=True, stop=True)
                o2_sb = ppool.tile([PT, 1], f32, name="o2_sb")
                nc.scalar.activation(o2_sb[:], o2_ps[:], AF.Copy, bias=bp2f)
                nc.sync.dma_start(outp[pc * PT:(pc + 1) * PT, :], o2_sb[:])

    nc.compile()
    return nc


def kernel(**inputs) -> np.ndarray:
    meta, shared, per_core = _preprocess(**inputs)
    key = tuple(sorted(meta.items()))
    if key not in _prog_cache:
        _prog_cache[key] = _build(meta)
    nc = _prog_cache[key]
    in_maps = [dict(shared, **per_core[k]) for k in range(NCORES)]
    res = bass_utils.run_bass_kernel_spmd(
        nc, in_maps, core_ids=list(range(NCORES)))
    out = np.concatenate(
        [np.asarray(res.results[k]["out"])[:, 0] for k in range(NCORES)])
    return out.astype(np.float32)


# revision 22
# speedup vs baseline: 1.0552x; 1.0034x over previous
"""Trainium2 Bass kernel for a 2-layer GCN + link predictor (PrimeKG drug
repurposing GNN).

Strategy (8 NeuronCores, SPMD single program):
  - Nodes are permuted into 128-node "buckets" balanced by in-degree; each
    core owns NBLK/8 consecutive buckets (rows of the aggregation).
  - Edges are grouped by destination bucket and padded to chunks of 128.
    segment_sum is computed per bucket as a sequence of PE matmuls:
       aggT[f, n] += sum_e xgath[e, f] * onehot[e, n]
    where xgath is an indirect-DMA gather of source-node features and
    onehot[e, n] = vals[e] * (n == local_row[e]) built on the vector engine.
  - x = node_emb + type_onehot.T @ type_emb is built sharded (original node
    order, host-precomputed transposed one-hot) and AllGathered; h and z
    live in permuted order, sharded and AllGathered likewise.
  - Pairs are sharded by batch; the predictor gathers z rows, transposes on
    the PE, and runs the tiny MLP per 128-pair chunk.

Feature tables and matmuls are fp16 with f32 PSUM accumulation.
"""

import numpy as np

import concourse.bass as bass
import concourse.bacc as bacc
import concourse.tile as tile
import concourse.mybir as mybir
from concourse import bass_utils

PT = 128  # partitions
NCORES = 8
TE = 16   # padded type-embedding rows

F16 = np.float16

_prog_cache: dict = {}


def _preprocess(node_type_ids, adj_rows, adj_cols, adj_vals, pairs,
                node_emb, type_emb, W1, b1, W2, b2, Wp1, bp1, Wp2, bp2):
    N, H = node_emb.shape
    T = type_emb.shape[0]
    E = adj_rows.shape[0]
    P2 = pairs.shape[1]
    D = W2.shape[1]
    assert H == PT and T <= TE and Wp1.shape == (3 * D, D)

    NPB = PT * NCORES
    NPAD = -(-N // NPB) * NPB
    NBLK = NPAD // PT
    BPC = NBLK // NCORES

    rows = np.asarray(adj_rows).astype(np.int64)
    cols = np.asarray(adj_cols).astype(np.int64)
    vals = np.asarray(adj_vals).astype(np.float32)
    types = np.asarray(node_type_ids).astype(np.int64)

    # Degree-balanced bucket assignment: deal nodes (sorted by in-degree
    # desc) round-robin across the NBLK buckets, then repair-swap nodes
    # between heavy and light buckets to pull the max bucket load down to
    # the next multiple-of-128 boundary.
    deg = np.bincount(rows, minlength=N).astype(np.int64)
    deg_pad = np.zeros(NPAD, np.int64)
    deg_pad[:N] = deg
    order = np.argsort(-deg_pad, kind="stable")
    i = np.arange(NPAD)
    bucket_of_rank = i % NBLK
    slot_of_rank = i // NBLK
    bucket_of = np.empty(NPAD, np.int64)
    bucket_of[order] = bucket_of_rank
    loads = np.bincount(bucket_of, weights=deg_pad, minlength=NBLK).astype(
        np.int64)
    target = max(PT, int(-(-int(loads.max()) // PT) - 1) * PT)
    members = [list(order[b::NBLK][::-1]) for b in range(NBLK)]  # asc degree
    for _ in range(2000):
        hb = int(np.argmax(loads))
        if loads[hb] <= target:
            break
        lb = int(np.argmin(loads))
        done = False
        for mi in range(len(members[hb]) - 1, -1, -1):
            m = members[hb][mi]
            for li, l in enumerate(members[lb]):
                delta = deg_pad[m] - deg_pad[l]
                if delta <= 0:
                    break
                if loads[lb] + delta <= target:
                    members[hb][mi], members[lb][li] = l, m
                    loads[hb] -= delta
                    loads[lb] += delta
                    done = True
                    break
            if done:
                break
        if not done:
            break
    perm = np.empty(N, np.int64)
    for b in range(NBLK):
        for s, m in enumerate(members[b]):
            if m < N:
                perm[m] = b * PT + s

    prow = perm[rows]
    bkt = prow // PT
    rid = (prow % PT).astype(np.float32)
    cnt = np.bincount(bkt, minlength=NBLK)
    SH = BPC * PT

    # Per-layer edge layout, local-first: chunk 0 of each bucket holds only
    # edges whose gather source row lives in the processing core's shard
    # (gathered from the shard table before the AllGather completes);
    # remaining edges go to chunks 1.. with full-table ids.
    def layer_arrays(tid):
        owner = bkt // BPC
        local = (tid // SH) == owner
        eo = np.lexsort((~local, bkt))
        b2 = bkt[eo]
        l2 = local[eo]
        t2 = tid[eo]
        r2 = rid[eo]
        v2 = vals[eo]
        cnts = np.bincount(b2, minlength=NBLK)
        st = np.concatenate([[0], np.cumsum(cnts)[:-1]])
        pos = np.arange(E) - st[b2]
        nl = np.bincount(b2, weights=l2.astype(np.float64),
                         minlength=NBLK).astype(np.int64)
        nl0 = np.minimum(nl, PT)
        slot = np.where(l2 & (pos < PT), pos, PT + pos - nl0[b2])
        owner2 = b2 // BPC
        ids = np.where(slot < PT, t2 - owner2 * SH, t2).astype(np.int32)
        Cl = int(-(-int((PT + cnts - nl0).max()) // PT))
        return b2, slot, ids, r2, v2, Cl

    b2a, slot_a, ids_a, rid_a, val_a, C1 = layer_arrays(cols)
    b2b, slot_b, ids_b, rid_b, val_b, C2 = layer_arrays(perm[cols])
    C = max(C1, C2)
    CAP = C * PT

    def fill(b2, slot, ids, r2, v2):
        ec = np.zeros((NBLK, CAP), np.int32)
        er = np.zeros((NBLK, CAP), np.float32)
        ev = np.zeros((NBLK, CAP), np.float32)
        ec[b2, slot] = ids
        er[b2, slot] = r2
        ev[b2, slot] = v2
        return ec, er, ev

    ecol1, erid1, eval1 = fill(b2a, slot_a, ids_a, rid_a, val_a)
    ecol2, erid2, eval2 = fill(b2b, slot_b, ids_b, rid_b, val_b)

    def per_core_T(a):
        # [NBLK, C*PT] -> per-core [PT, BPC*C]; column blk*C+c, partition p
        # holds bucket (core*BPC+blk) edge slot c*128+p.
        out = []
        for k in range(NCORES):
            sub = a[k * BPC:(k + 1) * BPC].reshape(BPC, C, PT)
            out.append(np.ascontiguousarray(
                sub.transpose(2, 0, 1).reshape(PT, BPC * C)))
        return out

    cols1_k = per_core_T(ecol1)
    rid1_k = per_core_T(erid1)
    val1_k = per_core_T(eval1)
    cols2_k = per_core_T(ecol2)
    rid2_k = per_core_T(erid2)
    val2_k = per_core_T(eval2)

    PPC = P2 // NCORES
    assert PPC % PT == 0
    PC = PPC // PT
    pp = perm[np.asarray(pairs).astype(np.int64)]
    psrc_k = [np.ascontiguousarray(
        pp[0, k * PPC:(k + 1) * PPC].reshape(PC, PT).T.astype(np.int32))
        for k in range(NCORES)]
    pdst_k = [np.ascontiguousarray(
        pp[1, k * PPC:(k + 1) * PPC].reshape(PC, PT).T.astype(np.int32))
        for k in range(NCORES)]

    types_pad = np.zeros(NPAD, np.int64)
    types_pad[:N] = types
    types_oh_t = np.zeros((TE, NPAD), F16)  # transposed one-hot, exact 0/1
    types_oh_t[types_pad, np.arange(NPAD)] = 1.0

    node_emb_pad = np.zeros((NPAD, H), F16)
    node_emb_pad[:N] = np.asarray(node_emb, np.float32).astype(F16)
    type_emb_pad = np.zeros((TE, H), F16)
    type_emb_pad[:T] = np.asarray(type_emb, np.float32).astype(F16)

    SH = BPC * PT  # x-shard rows per core
    Wp1 = np.asarray(Wp1, np.float32)
    shared = dict(
        type_emb=type_emb_pad,
        w1=np.asarray(W1, np.float32).astype(F16),
        w2=np.asarray(W2, np.float32).astype(F16),
        wp1a=np.ascontiguousarray(Wp1[0:D]).astype(F16),
        wp1b=np.ascontiguousarray(Wp1[D:2 * D]).astype(F16),
        wp1c=np.ascontiguousarray(Wp1[2 * D:3 * D]).astype(F16),
        wp2=np.asarray(Wp2, np.float32).astype(F16),
        b1bc=np.ascontiguousarray(np.broadcast_to(
            np.asarray(b1, np.float32), (PT, H))),
        b2bc=np.ascontiguousarray(np.broadcast_to(
            np.asarray(b2, np.float32), (PT, D))),
        bp1col=np.asarray(bp1, np.float32).reshape(D, 1),
        iota=np.ascontiguousarray(np.broadcast_to(
            np.arange(PT, dtype=np.float32), (PT, PT))).astype(F16),
        ident=np.eye(PT, dtype=F16),
    )
    per_core = [dict(cols1=cols1_k[k], cols2=cols2_k[k], rid1=rid1_k[k],
                     val1=val1_k[k], rid2=rid2_k[k], val2=val2_k[k],
                     psrc=psrc_k[k], pdst=pdst_k[k],
                     node_emb=np.ascontiguousarray(
                         node_emb_pad[k * SH:(k + 1) * SH]),
                     types_oh=np.ascontiguousarray(
                         types_oh_t[:, k * SH:(k + 1) * SH]))
                for k in range(NCORES)]
    meta = dict(NPAD=NPAD, NBLK=NBLK, BPC=BPC, C=C, PC=PC, H=H, D=D,
                bp2f=float(np.asarray(bp2).reshape(-1)[0]))
    return meta, shared, per_core


def _build(meta):
    NPAD, NBLK, BPC, C, PC = (meta["NPAD"], meta["NBLK"], meta["BPC"],
                              meta["C"], meta["PC"])
    H, D, bp2f = meta["H"], meta["D"], meta["bp2f"]
    f32, f16, i32 = mybir.dt.float32, mybir.dt.float16, mybir.dt.int32
    AF = mybir.ActivationFunctionType
    Alu = mybir.AluOpType
    RG = [list(range(NCORES))]
    SH = BPC * PT

    nc = bacc.Bacc("TRN2", target_bir_lowering=False, debug=False,
                   num_devices=NCORES, dynamic_dma_scratch_size=32768,
                   num_swdge_queues=2)

    # kernel I/O
    node_emb = nc.dram_tensor("node_emb", [SH, H], f16, kind="ExternalInput")
    type_emb = nc.dram_tensor("type_emb", [TE, H], f16, kind="ExternalInput")
    tyoh_d = nc.dram_tensor("types_oh", [TE, SH], f16, kind="ExternalInput")
    cols1_d = nc.dram_tensor("cols1", [PT, BPC * C], i32, kind="ExternalInput")
    cols2_d = nc.dram_tensor("cols2", [PT, BPC * C], i32, kind="ExternalInput")
    rid1_d = nc.dram_tensor("rid1", [PT, BPC * C], f32, kind="ExternalInput")
    val1_d = nc.dram_tensor("val1", [PT, BPC * C], f32, kind="ExternalInput")
    rid2_d = nc.dram_tensor("rid2", [PT, BPC * C], f32, kind="ExternalInput")
    val2_d = nc.dram_tensor("val2", [PT, BPC * C], f32, kind="ExternalInput")
    psrc_d = nc.dram_tensor("psrc", [PT, PC], i32, kind="ExternalInput")
    pdst_d = nc.dram_tensor("pdst", [PT, PC], i32, kind="ExternalInput")
    w1_d = nc.dram_tensor("w1", [H, H], f16, kind="ExternalInput")
    w2_d = nc.dram_tensor("w2", [H, D], f16, kind="ExternalInput")
    wp1a_d = nc.dram_tensor("wp1a", [D, D], f16, kind="ExternalInput")
    wp1b_d = nc.dram_tensor("wp1b", [D, D], f16, kind="ExternalInput")
    wp1c_d = nc.dram_tensor("wp1c", [D, D], f16, kind="ExternalInput")
    wp2_d = nc.dram_tensor("wp2", [D, 1], f16, kind="ExternalInput")
    b1bc_d = nc.dram_tensor("b1bc", [PT, H], f32, kind="ExternalInput")
    b2bc_d = nc.dram_tensor("b2bc", [PT, D], f32, kind="ExternalInput")
    bp1c_d = nc.dram_tensor("bp1col", [D, 1], f32, kind="ExternalInput")
    iota_d = nc.dram_tensor("iota", [PT, PT], f16, kind="ExternalInput")
    ident_d = nc.dram_tensor("ident", [PT, PT], f16, kind="ExternalInput")
    outp = nc.dram_tensor("out", [PC * PT, 1], f32, kind="ExternalOutput")

    # internal feature tables
    x_shard = nc.dram_tensor("x_shard", [SH, H], f16, kind="Internal")
    x_full = nc.dram_tensor("x_full", [NPAD, H], f16, kind="Internal",
                            addr_space="Shared")
    h_shard = nc.dram_tensor("h_shard", [SH, H], f16, kind="Internal")
    h_full = nc.dram_tensor("h_full", [NPAD, H], f16, kind="Internal",
                            addr_space="Shared")
    z_shard = nc.dram_tensor("z_shard", [SH, D], f16, kind="Internal")
    z_full = nc.dram_tensor("z_full", [NPAD, D], f16, kind="Internal",
                            addr_space="Shared")

    with tile.TileContext(nc) as tc:
        with (
            tc.tile_pool(name="const", bufs=1) as cpool,
            tc.tile_pool(name="idx", bufs=1) as ipool,
            tc.tile_pool(name="xne", bufs=3) as xnp,
            tc.tile_pool(name="gath", bufs=12) as gpool,
            tc.tile_pool(name="onep", bufs=8) as opool,
            tc.tile_pool(name="accs", bufs=3) as apool,
            tc.tile_pool(name="outs", bufs=3) as hpool,
            tc.tile_pool(name="pred", bufs=6) as ppool,
            tc.tile_pool(name="locp", bufs=BPC) as locpool,
            tc.tile_pool(name="ps_agg", bufs=3, space="PSUM") as ps_agg,
            tc.tile_pool(name="ps_out", bufs=3, space="PSUM") as ps_out,
            tc.tile_pool(name="ps_tr", bufs=2, space="PSUM") as ps_tr,
        ):
            def sb(pool, dram, shape, dtype):
                t = pool.tile(shape, dtype, name=dram.name + "_sb")
                nc.sync.dma_start(t[:], dram[:])
                return t

            # resident SBUF state
            iota_sb = sb(cpool, iota_d, [PT, PT], f16)
            ident_sb = sb(cpool, ident_d, [PT, PT], f16)
            w1_sb = sb(cpool, w1_d, [H, H], f16)
            w2_sb = sb(cpool, w2_d, [H, D], f16)
            wp1a_sb = sb(cpool, wp1a_d, [D, D], f16)
            wp1b_sb = sb(cpool, wp1b_d, [D, D], f16)
            wp1c_sb = sb(cpool, wp1c_d, [D, D], f16)
            wp2_sb = sb(cpool, wp2_d, [D, 1], f16)
            b1bc_sb = sb(cpool, b1bc_d, [PT, H], f32)
            b2bc_sb = sb(cpool, b2bc_d, [PT, D], f32)
            bp1c_sb = sb(cpool, bp1c_d, [D, 1], f32)
            tyemb_sb = sb(cpool, type_emb, [TE, H], f16)
            tyoh_sb = sb(ipool, tyoh_d, [TE, SH], f16)
            cols1_sb = sb(ipool, cols1_d, [PT, BPC * C], i32)
            cols2_sb = sb(ipool, cols2_d, [PT, BPC * C], i32)
            rid1_sb = sb(ipool, rid1_d, [PT, BPC * C], f32)
            val1_sb = sb(ipool, val1_d, [PT, BPC * C], f32)
            rid2_sb = sb(ipool, rid2_d, [PT, BPC * C], f32)
            val2_sb = sb(ipool, val2_d, [PT, BPC * C], f32)
            psrc_sb = sb(ipool, psrc_d, [PT, PC], i32)
            pdst_sb = sb(ipool, pdst_d, [PT, PC], i32)

            # ---- Phase X (sharded): x = node_emb + types_oh.T @ type_emb ----
            SB = 7 if BPC % 7 == 0 else 1  # blocks per supertile
            assert BPC % SB == 0
            for st in range(BPC // SB):
                a = st * SB * PT
                ne = xnp.tile([PT, SB * H], f16, name="ne")
                nc.sync.dma_start(
                    ne[:].rearrange("p (b f) -> p b f", b=SB),
                    node_emb[a:a + SB * PT, :].rearrange(
                        "(b p) f -> p b f", p=PT))
                xs = xnp.tile([PT, SB * H], f16, name="xs")
                for b in range(SB):
                    blk = st * SB + b
                    t_ps = ps_out.tile([PT, H], f32, name="o_ps", tag="o_ps")
                    nc.tensor.matmul(
                        t_ps[:], lhsT=tyoh_sb[:, blk * PT:(blk + 1) * PT],
                        rhs=tyemb_sb[:], start=True, stop=True)
                    nc.vector.tensor_add(
                        xs[:, b * H:(b + 1) * H], ne[:, b * H:(b + 1) * H],
                        t_ps[:])
                nc.sync.dma_start(
                    x_shard[a:a + SB * PT, :].rearrange("(b p) f -> p b f", p=PT),
                    xs[:].rearrange("p (b f) -> p b f", b=SB))
            nc.gpsimd.collective_compute(
                "AllGather", Alu.bypass, replica_groups=RG,
                ins=[x_shard[:]], outs=[x_full[:]])

            # ---- GCN layer ----
            # Chunk 0 of every bucket holds core-local edges: gathered from
            # the (pre-AllGather) shard table into a dedicated per-bucket
            # buffer so these DMAs can fill the AllGather barrier window.
            def gcn_layer(shard_table, full_table, colsb, ridb, valb,
                          dst_shard, w_sb, bias_sb, Dout, relu, suffix):
                locs = []
                for blk in range(BPC):
                    lg = locpool.tile([PT, H], f16, name="loc" + suffix)
                    bi = nc.gpsimd.indirect_dma_start(
                        out=lg[:], out_offset=None, in_=shard_table[:],
                        in_offset=bass.IndirectOffsetOnAxis(
                            ap=colsb[:, blk * C:blk * C + 1], axis=0))
                    if blk % 2:
                        bi.ins.queue = "qPoolDynamic1"
                    locs.append(lg)
                for blk in range(BPC):
                    agg_ps = ps_agg.tile([PT, PT], f32, name="agg_ps")
                    for c in range(C):
                        g = blk * C + c
                        if c == 0:
                            xg = locs[blk]
                        else:
                            xg = gpool.tile([PT, H], f16, name="xg" + suffix)
                            bi = nc.gpsimd.indirect_dma_start(
                                out=xg[:], out_offset=None, in_=full_table[:],
                                in_offset=bass.IndirectOffsetOnAxis(
                                    ap=colsb[:, g:g + 1], axis=0))
                            if c % 2:
                                bi.ins.queue = "qPoolDynamic1"
                        oh = opool.tile([PT, PT], f16, name="oh" + suffix)
                        nc.vector.tensor_scalar(
                            oh[:], iota_sb[:], ridb[:, g:g + 1],
                            valb[:, g:g + 1], op0=Alu.is_equal, op1=Alu.mult)
                        nc.tensor.matmul(agg_ps[:], lhsT=xg[:], rhs=oh[:],
                                         start=(c == 0), stop=(c == C - 1))
                    aggT_sb = apool.tile([PT, PT], f16, name="aggT" + suffix)
                    nc.vector.tensor_copy(aggT_sb[:], agg_ps[:])
                    o_ps = ps_out.tile([PT, Dout], f32, name="o_ps", tag="o_ps")
                    nc.tensor.matmul(o_ps[:], lhsT=aggT_sb[:], rhs=w_sb[:],
                                     start=True, stop=True)
                    o_sb = hpool.tile([PT, Dout], f16, name="osb" + suffix)
                    if relu:
                        ob = hpool.tile([PT, Dout], f32, name="ob" + suffix)
                        nc.vector.tensor_add(ob[:], o_ps[:], bias_sb[:])
                        nc.scalar.activation(o_sb[:], ob[:], AF.Relu)
                    else:
                        nc.vector.tensor_add(o_sb[:], o_ps[:], bias_sb[:])
                    nc.sync.dma_start(
                        dst_shard[blk * PT:(blk + 1) * PT, :], o_sb[:])

            gcn_layer(x_shard, x_full, cols1_sb, rid1_sb, val1_sb,
                      h_shard, w1_sb, b1bc_sb, H, relu=True, suffix="1")
            nc.gpsimd.collective_compute(
                "AllGather", Alu.bypass, replica_groups=RG,
                ins=[h_shard[:]], outs=[h_full[:]])

            gcn_layer(h_shard, h_full, cols2_sb, rid2_sb, val2_sb,
                      z_shard, w2_sb, b2bc_sb, D, relu=False, suffix="2")
            nc.gpsimd.collective_compute(
                "AllGather", Alu.bypass, replica_groups=RG,
                ins=[z_shard[:]], outs=[z_full[:]])

            # ---- predictor ----
            for pc in range(PC):
                sg = ppool.tile([PT, D], f16, name="sg")
                nc.gpsimd.indirect_dma_start(
                    out=sg[:], out_offset=None, in_=z_full[:],
                    in_offset=bass.IndirectOffsetOnAxis(
                        ap=psrc_sb[:, pc:pc + 1], axis=0))
                dg = ppool.tile([PT, D], f16, name="dg")
                nc.gpsimd.indirect_dma_start(
                    out=dg[:], out_offset=None, in_=z_full[:],
                    in_offset=bass.IndirectOffsetOnAxis(
                        ap=pdst_sb[:, pc:pc + 1], axis=0))
                sgt_ps = ps_tr.tile([D, PT], f16, name="sgt_ps", tag="tps")
                nc.tensor.transpose(sgt_ps[:], sg[:], ident_sb[:])
                dgt_ps = ps_tr.tile([D, PT], f16, name="dgt_ps", tag="tps")
                nc.tensor.transpose(dgt_ps[:], dg[:], ident_sb[:])
                sgt = ppool.tile([D, PT], f16, name="sgt")
                nc.vector.tensor_copy(sgt[:], sgt_ps[:])
                dgt = ppool.tile([D, PT], f16, name="dgt")
                nc.vector.tensor_copy(dgt[:], dgt_ps[:])
                sdt = ppool.tile([D, PT], f16, name="sdt")
                nc.vector.tensor_mul(sdt[:], sgt[:], dgt[:])
                yt_ps = ps_agg.tile([D, PT], f32, name="yt_ps", tag="agg_ps")
                nc.tensor.matmul(yt_ps[:], lhsT=wp1a_sb[:], rhs=sgt[:],
                                 start=True, stop=False)
                nc.tensor.matmul(yt_ps[:], lhsT=wp1b_sb[:], rhs=dgt[:],
                                 start=False, stop=False)
                nc.tensor.matmul(yt_ps[:], lhsT=wp1c_sb[:], rhs=sdt[:],
                                 start=False, stop=True)
                r_sb = ppool.tile([D, PT], f16, name="r_sb")
                nc.scalar.activation(r_sb[:], yt_ps[:], AF.Relu,
                                     bias=bp1c_sb[:])
                o2_ps = ps_out.tile([PT, 1], f32, name="o2_ps", tag="o_ps")
                nc.tensor.matmul(o2_ps[:], lhsT=r_sb[:], rhs=wp2_sb[:],
                                 start=True, stop=True)
                o2_sb = ppool.tile([PT, 1], f32, name="o2_sb")
                nc.scalar.activation(o2_sb[:], o2_ps[:], AF.Copy, bias=bp2f)
                nc.sync.dma_start(outp[pc * PT:(pc + 1) * PT, :], o2_sb[:])

    nc.compile()
    return nc


def kernel(**inputs) -> np.ndarray:
    meta, shared, per_core = _preprocess(**inputs)
    key = tuple(sorted(meta.items()))
    if key not in _prog_cache:
        _prog_cache[key] = _build(meta)
    nc = _prog_cache[key]
    in_maps = [dict(shared, **per_core[k]) for k in range(NCORES)]
    res = bass_utils.run_bass_kernel_spmd(
        nc, in_maps, core_ids=list(range(NCORES)))
    out = np.concatenate(
        [np.asarray(res.results[k]["out"])[:, 0] for k in range(NCORES)])
    return out.astype(np.float32)
